# revision 2
# baseline (speedup 1.0000x reference)
"""3-layer 2-head GAT + BatchNorm/ReLU + per-graph max-pool + FC on 8 trn2 NeuronCores.

Sharding: graph/data-parallel over dst nodes. Host relabels nodes graph-major
(degree-profile sorted within each graph, padded to graph-aligned 128-node
tiles), packs graphs onto cores, and builds per-core ELL gather schedules with
a cross-core-uniform static shape (all per-core variation is data).

Device per layer: bf16 table rows [in_features | al_src] (256B rows, one row
per node id) are gathered per edge-slot with dma_gather (int16 indices =>
32768-row windows); attention scores/softmax and the weighted feature sum run
on the vector/scalar engines along the ELL free axis; the conv matmul
(agg @ W) runs per-tile on the PE after a transpose. BN stats go through a
tiny AllReduce; next-layer features are replicated with an AllGather of bf16
shard rows; the final per-graph max pool uses an AllReduce(max).

Dispatch: a cached jit(shard_map) executable with the large call-invariant
inputs (gather schedules, masks, pool bias) resident on device; per call only
the node features (per-core [T,P,4] shard) and the small weight tensors are
uploaded. The layer-1 feature table is built on device from the x shard
(transpose+matmul for the attention logits, zero-padded bf16 rows) and
replicated with the same AllGather the later layers use.
"""
import os
import numpy as np
import ml_dtypes

import jax
import concourse.bass as bass
import concourse.bacc as bacc
import concourse.mybir as mybir
import concourse.tile as tile
from concourse._compat import cdiv, get_trn_type
from concourse.library_config import mlp

P = 128
NCORES = 8
NEG = -1.0e30
EPS = 1e-5
f32 = mybir.dt.float32
bf16 = mybir.dt.bfloat16
i16 = mybir.dt.int16
AF = mybir.ActivationFunctionType
OP = mybir.AluOpType

# (layer, Fin, C-per-head, HC=2C)
LAYERS = [(1, 3, 16, 32), (2, 32, 32, 64), (3, 64, 64, 128)]

# call-invariant inputs (functions of edge_index/batch/graph structure only):
# uploaded to device once per preprocessing cache entry.
STATIC_NAMES = ("gidx1", "gidx2", "gidx3", "nodemask", "negb", "poolbias",
                "ident")


def wrap_idxs(flat):
    n = len(flat)
    assert n % 16 == 0
    a = flat.astype(np.int16).reshape(n // 16, 16).T
    return np.ascontiguousarray(np.tile(a, (8, 1)))


# ----------------------------------------------------------------------------
# host preprocessing
# ----------------------------------------------------------------------------

def preprocess(x, edge_index, batch, G, SUPS, CH):
    N = x.shape[0]
    src = np.concatenate([edge_index[0], np.arange(N)]).astype(np.int64)
    dst = np.concatenate([edge_index[1], np.arange(N)]).astype(np.int64)
    batch = np.asarray(batch).astype(np.int64)
    counts = np.bincount(batch, minlength=G)
    gstart = np.concatenate([[0], np.cumsum(counts)])

    nw0 = max(cdiv(N, CH), 1)
    prof = np.zeros((N, nw0), np.int64)
    np.add.at(prof, (dst, src // CH), 1)
    order = np.empty(N, np.int64)
    for g in range(G):
        s, e = int(gstart[g]), int(gstart[g + 1])
        idx = np.arange(s, e)
        key = np.lexsort(tuple(prof[s:e, c] for c in range(nw0 - 1, -1, -1)))
        order[s:e] = idx[key]

    tiles = []
    for g in range(G):
        s, e = int(gstart[g]), int(gstart[g + 1])
        ids = order[s:e]
        nt = cdiv(max(len(ids), 1), P)
        pad = np.full(nt * P, -1, np.int64)
        pad[: len(ids)] = ids
        for t in range(nt):
            tiles.append((g, pad[t * P : (t + 1) * P]))

    by_g = {}
    for g, arr in tiles:
        by_g.setdefault(g, []).append(arr)
    core_tiles = [[] for _ in range(NCORES)]
    loads = [0] * NCORES
    for g in sorted(by_g, key=lambda g: -len(by_g[g])):
        j = int(np.argmin(loads))
        for arr in by_g[g]:
            core_tiles[j].append((g, arr))
        loads[j] += len(by_g[g])
    SUPMAX = max(SUPS.values())
    T = cdiv(max(loads), SUPMAX) * SUPMAX
    for j in range(NCORES):
        while len(core_tiles[j]) < T:
            core_tiles[j].append((-1, np.full(P, -1, np.int64)))
    NT = NCORES * T * P
    NWIN = cdiv(NT, CH)

    def assign(ct):
        o2n = np.full(N, -1, np.int64)
        n2o = np.full(NT, -1, np.int64)
        for j in range(NCORES):
            for i, (g, arr) in enumerate(ct[j]):
                base = (j * T + i) * P
                r = arr >= 0
                o2n[arr[r]] = base + np.nonzero(r)[0]
                n2o[base : base + P][r] = arr[r]
        return o2n, n2o

    def k_of(o2n):
        sn, dn = o2n[src], o2n[dst]
        cnt = np.bincount(dn * NWIN + sn // CH, minlength=NT * NWIN).reshape(NT, NWIN)
        return sn, dn, cnt.reshape(NCORES, T, P, NWIN).max(axis=2)

    o2n, n2o = assign(core_tiles)

    # Re-sort nodes within each graph by their FINAL-window in-degree profile
    # (graph->core packing moved graphs to entirely different id windows, so
    # the original-order profile sort is stale). Converges in one iteration:
    # within-graph reordering moves ids by < graph size << window size.
    for _ in range(2):
        prof2 = np.zeros((N, NWIN), np.int64)
        sn = o2n[src]
        np.add.at(prof2, (dst, sn // CH), 1)
        for j in range(NCORES):
            by_graph = {}
            for i, (g, arr) in enumerate(core_tiles[j]):
                by_graph.setdefault(g, []).append(i)
            for g, idxs_t in by_graph.items():
                if g < 0:
                    continue
                ids = np.concatenate([core_tiles[j][i][1] for i in idxs_t])
                ids = ids[ids >= 0]
                key = np.lexsort(tuple(prof2[ids, c]
                                       for c in range(NWIN - 1, -1, -1)))
                ids = ids[key]
                pad = np.full(len(idxs_t) * P, -1, np.int64)
                pad[: len(ids)] = ids
                for n_i, i in enumerate(idxs_t):
                    core_tiles[j][i] = (g, pad[n_i * P : (n_i + 1) * P])
        o2n, n2o = assign(core_tiles)

    _, _, Kc = k_of(o2n)
    for j in range(NCORES):
        key = np.argsort(-Kc[j].sum(axis=1), kind="stable")
        core_tiles[j] = [core_tiles[j][i] for i in key]
    o2n, n2o = assign(core_tiles)
    snew, dnew, Kc = k_of(o2n)
    Kuni = Kc.max(axis=0)  # [T, NWIN]

    layer_K = {}
    for L, SUP in SUPS.items():
        ns = T // SUP
        K = np.zeros((ns, NWIN), np.int64)
        for s in range(ns):
            K[s] = Kuni[s * SUP : (s + 1) * SUP].max(axis=0)
        layer_K[L] = K

    filler = np.nonzero(n2o < 0)[0]
    pad_row = {}
    for c in range(NWIN):
        f = filler[(filler // CH) == c]
        assert len(f) > 0, f"no filler row in window {c}"
        pad_row[c] = int(f[0])

    schunk = snew // CH
    eo = np.lexsort((snew, schunk, dnew))
    ds, cs, ss = dnew[eo], schunk[eo], snew[eo]
    grp = ds * NWIN + cs
    firsts = np.ones(len(grp), bool)
    firsts[1:] = grp[1:] != grp[:-1]
    g0 = np.nonzero(firsts)[0]
    rank = np.arange(len(grp)) - np.repeat(g0, np.diff(np.concatenate([g0, [len(grp)]])))
    KMAX = max(int(Kuni.max()), 1)
    slot = np.full((NT, NWIN, KMAX), -1, np.int64)
    slot[ds, cs, rank] = ss

    data = [dict() for _ in range(NCORES)]
    gcols = {}
    for L, SUP in SUPS.items():
        K = layer_K[L]
        ns = T // SUP
        for j in range(NCORES):
            segs = []
            for s in range(ns):
                for c in range(NWIN):
                    k = int(K[s, c])
                    if k == 0:
                        continue
                    rows = slot[(j * T + s * SUP) * P : (j * T + (s + 1) * SUP) * P, c, :k]
                    sub = rows.reshape(SUP, P, k)
                    loc = np.where(sub < 0, pad_row[c], sub) - c * CH
                    assert loc.min() >= 0 and loc.max() < CH
                    segs.append(wrap_idxs(loc.transpose(0, 2, 1).reshape(-1)))
            arr = (np.concatenate(segs, axis=1) if segs
                   else np.zeros((P, 8), np.int16))
            data[j][f"gidx{L}"] = np.ascontiguousarray(arr)
        gcols[L] = data[0][f"gidx{L}"].shape[1]
        for j in range(NCORES):
            assert data[j][f"gidx{L}"].shape[1] == gcols[L]

    for j in range(NCORES):
        nm = np.zeros((T, P), np.float32)
        pb = np.full((G, T), NEG, np.float32)
        for i, (g, arr) in enumerate(core_tiles[j]):
            nm[i] = (arr >= 0).astype(np.float32)
            if g >= 0:
                pb[g, i] = 0.0
        data[j]["nodemask"] = nm
        data[j]["negb"] = ((nm - 1.0) * 1e30).astype(np.float32)
        data[j]["poolbias"] = np.ascontiguousarray(
            np.repeat(pb[:, None, :], P, axis=1).reshape(G * P, T))
        data[j]["ident"] = np.eye(128, dtype=np.float32)

    cfg = dict(N=int(N), G=int(G), T=int(T), NT=int(NT), NWIN=int(NWIN), CH=int(CH),
               SUPS=dict(SUPS), gcols=gcols,
               layer_K={L: K.astype(int) for L, K in layer_K.items()})
    aux = dict(o2n=o2n, n2o=n2o)
    return cfg, data, aux


def host_params(x, prm, cfg, data, aux):
    """Per-call (x/param-dependent) inputs: per-core x shard + small weights."""
    n2o = aux["n2o"]
    T, NT = cfg["T"], cfg["NT"]

    def avec(W, a_s, a_d, F, Hc):
        Wr = np.asarray(W).reshape(F, 2, Hc)
        vs = np.stack([Wr[:, h, :] @ np.asarray(a_s)[h] for h in range(2)], axis=1)
        vd = np.stack([Wr[:, h, :] @ np.asarray(a_d)[h] for h in range(2)], axis=1)
        return np.concatenate([vs, vd], axis=1).astype(np.float32)

    av1 = avec(prm["W1"], prm["as1"], prm["ad1"], 3, 16)
    av1p = np.zeros((4, 4), np.float32)
    av1p[0:3] = av1
    av2 = avec(prm["W2"], prm["as2"], prm["ad2"], 32, 32)
    av3 = avec(prm["W3"], prm["as3"], prm["ad3"], 64, 64)

    shared = {
        "w1": np.asarray(prm["W1"], np.float32),
        "w2": np.asarray(prm["W2"], np.float32),
        "w3": np.asarray(prm["W3"], np.float32),
        "avec1": av1p, "avec2": av2, "avec3": av3,
        "fcw": np.asarray(prm["fcw"], np.float32),
        "fcb": np.tile(np.asarray(prm["fcb"], np.float32).reshape(1, -1), (cfg["G"], 1)),
    }
    for L, F, Hc, HC in LAYERS:
        shared[f"gam{L}"] = np.asarray(prm[f"g{L}"], np.float32).reshape(1, -1)
        shared[f"bet{L}"] = np.asarray(prm[f"be{L}"], np.float32).reshape(1, -1)
    xr = np.asarray(x, np.float32)
    dyn = [dict(shared) for _ in range(NCORES)]
    for j in range(NCORES):
        ids = n2o[j * T * P : (j + 1) * T * P]
        xs = np.zeros((T * P, 4), np.float32)
        r = ids >= 0
        xs[r, 0:3] = xr[ids[r]]
        dyn[j]["xs"] = xs.reshape(T, P, 4)
    return dyn


# ----------------------------------------------------------------------------
# device program
# ----------------------------------------------------------------------------

def build(cfg):
    T, NT, NWIN, G, N, CH = (cfg["T"], cfg["NT"], cfg["NWIN"], cfg["G"],
                             cfg["N"], cfg["CH"])
    SUPS, layer_K, gcols = cfg["SUPS"], cfg["layer_K"], cfg["gcols"]
    core_ids = list(range(NCORES))

    nc = bacc.Bacc(get_trn_type() or "TRN2", target_bir_lowering=False)

    xs_t = nc.dram_tensor("xs", [T, P, 4], f32, kind="ExternalInput")
    gidx_t = {L: nc.dram_tensor(f"gidx{L}", [P, gcols[L]], i16, kind="ExternalInput")
              for L, _, _, _ in LAYERS}
    nmask_t = nc.dram_tensor("nodemask", [T, P], f32, kind="ExternalInput")
    negb_t = nc.dram_tensor("negb", [T, P], f32, kind="ExternalInput")
    poolb_t = nc.dram_tensor("poolbias", [G * P, T], f32, kind="ExternalInput")
    w_t = {1: nc.dram_tensor("w1", [3, 32], f32, kind="ExternalInput"),
           2: nc.dram_tensor("w2", [32, 64], f32, kind="ExternalInput"),
           3: nc.dram_tensor("w3", [64, 128], f32, kind="ExternalInput")}
    av_t = {1: nc.dram_tensor("avec1", [4, 4], f32, kind="ExternalInput"),
            2: nc.dram_tensor("avec2", [32, 4], f32, kind="ExternalInput"),
            3: nc.dram_tensor("avec3", [64, 4], f32, kind="ExternalInput")}
    gam_t = {L: nc.dram_tensor(f"gam{L}", [1, HC], f32, kind="ExternalInput")
             for L, F, Hc, HC in LAYERS}
    bet_t = {L: nc.dram_tensor(f"bet{L}", [1, HC], f32, kind="ExternalInput")
             for L, F, Hc, HC in LAYERS}
    fcw_t = nc.dram_tensor("fcw", [128, 10], f32, kind="ExternalInput")
    fcb_t = nc.dram_tensor("fcb", [G, 10], f32, kind="ExternalInput")
    ident_t = nc.dram_tensor("ident", [P, P], f32, kind="ExternalInput")
    out_t = nc.dram_tensor("out", [G, 10], f32, kind="ExternalOutput")

    # internal DRAM
    tabn = {1: nc.dram_tensor("tabA", [NT, 128], bf16, addr_space="Shared"),
            2: nc.dram_tensor("tabB", [NT, 128], bf16, addr_space="Shared"),
            3: nc.dram_tensor("tabC", [NT, 128], bf16, addr_space="Shared")}
    shard = {1: nc.dram_tensor("shardA", [T * P, 128], bf16),
             2: nc.dram_tensor("shardB", [T * P, 128], bf16),
             3: nc.dram_tensor("shardC", [T * P, 128], bf16)}
    conv_t = {L: nc.dram_tensor(f"conv{L}", [T, P, HC], f32)
              for L, F, Hc, HC in LAYERS}
    stin = {L: nc.dram_tensor(f"stin{L}", [1, 2 * HC], f32)
            for L, F, Hc, HC in LAYERS}
    stout = {L: nc.dram_tensor(f"stout{L}", [1, 2 * HC], f32, addr_space="Shared")
             for L, F, Hc, HC in LAYERS}
    poolin = nc.dram_tensor("poolin", [P, G], f32)
    poolout = nc.dram_tensor("poolout", [P, G], f32, addr_space="Shared")

    with tile.TileContext(nc) as tc:
        with (
            tc.tile_pool(name="persist", bufs=1) as pp,
            tc.tile_pool(name="io", bufs=int(os.environ.get("GAT_IOBUFS", "3"))) as iop,
            tc.tile_pool(name="gath", bufs=2) as gpool,
            tc.tile_pool(name="work", bufs=int(os.environ.get("GAT_WPBUFS", "2"))) as wp,
            tc.tile_pool(name="psum", bufs=2, space="PSUM") as psp,
        ):
            nc.gpsimd.load_library(mlp)
            tc.strict_bb_all_engine_barrier()
            ident = pp.tile([P, P], f32, tag="ident", name="ident")
            nc.sync.dma_start(ident[:], ident_t[:])
            ones = pp.tile([P, 1], f32, tag="ones", name="ones")
            nc.vector.memset(ones[:], 1.0)
            onesr = pp.tile([1, P], f32, tag="onesr", name="onesr")
            nc.vector.memset(onesr[:], 1.0)

            wsb = {}
            for L, F, Hc, HC in LAYERS:
                wsb[L] = pp.tile([F, HC], f32, tag=f"w{L}", name=f"w{L}")
                nc.sync.dma_start(wsb[L][:], w_t[L][:])
            avsb = {}
            for L in (1, 2, 3):
                Fin = 4 if L == 1 else LAYERS[L - 1][1]
                avsb[L] = pp.tile([Fin, 4], f32, tag=f"av{L}", name=f"av{L}")
                nc.sync.dma_start(avsb[L][:], av_t[L][:])
            gamsb, betsb = {}, {}
            for L, F, Hc, HC in LAYERS:
                gamsb[L] = pp.tile([1, HC], f32, tag=f"gam{L}", name=f"gam{L}")
                betsb[L] = pp.tile([1, HC], f32, tag=f"bet{L}", name=f"bet{L}")
                nc.sync.dma_start(gamsb[L][:], gam_t[L][:])
                nc.sync.dma_start(betsb[L][:], bet_t[L][:])
            fcwsb = pp.tile([128, 10], f32, tag="fcw", name="fcw")
            nc.sync.dma_start(fcwsb[:], fcw_t[:])
            fcbsb = pp.tile([G, 10], f32, tag="fcb", name="fcb")
            nc.sync.dma_start(fcbsb[:], fcb_t[:])
            poolbsb = pp.tile([P, G, T], f32, tag="poolb", name="poolb")
            nc.sync.dma_start(poolbsb[:], poolb_t[:].rearrange("(g p) t -> p g t", p=P))
            aldbuf = {L: pp.tile([P, T, 2], f32, tag=f"ald{L}", name=f"ald{L}")
                      for L in (1, 2, 3)}
            tmax = pp.tile([P, T], f32, tag="tmax", name="tmax")
            pool_sb = pp.tile([P, G], f32, tag="pool", name="pool")

            # ============ layer-1 table build (shard rows from x) ============
            SUP1 = SUPS[1]
            for s in range(T // SUP1):
                xst = iop.tile([P, SUP1, 4], f32, tag="xst", name="xst")
                nc.sync.dma_start(
                    xst[:], xs_t[s * SUP1 : (s + 1) * SUP1].rearrange("t p f -> p t f"))
                mkb = iop.tile([P, SUP1], f32, tag="mkb0", name="mkb0")
                nc.sync.dma_start(
                    mkb[:], nmask_t[s * SUP1 : (s + 1) * SUP1].rearrange("t p -> p t"))
                ngb = iop.tile([P, SUP1], f32, tag="ngb0", name="ngb0")
                nc.sync.dma_start(
                    ngb[:], negb_t[s * SUP1 : (s + 1) * SUP1].rearrange("t p -> p t"))
                rbc = wp.tile([P, SUP1, 128], bf16, tag="rbc", name="rbc")
                nc.vector.memset(rbc[:], 0.0)
                nc.vector.tensor_copy(out=rbc[:, :, 0:3], in_=xst[:, :, 0:3])
                for t in range(SUP1):
                    ti = s * SUP1 + t
                    tp0 = psp.tile([4, P], f32, tag="tp", name="tp0")
                    nc.tensor.transpose(out=tp0[:], in_=xst[:, t, :],
                                        identity=ident[:])
                    xT = wp.tile([4, P], f32, tag="xT", name="xT")
                    nc.vector.tensor_copy(out=xT[:], in_=tp0[:])
                    ps4 = psp.tile([P, 4], f32, tag="ps40", name="ps40", bufs=1)
                    nc.tensor.matmul(out=ps4[:], lhsT=xT[:], rhs=avsb[1][:],
                                     start=True, stop=True)
                    alsb = wp.tile([P, 2], f32, tag="alsb0", name="alsb0")
                    nc.vector.scalar_tensor_tensor(
                        out=alsb[:], in0=ps4[:, 0:2],
                        scalar=mkb[:, t : t + 1],
                        in1=ngb[:, t : t + 1].to_broadcast([P, 2]),
                        op0=OP.mult, op1=OP.add)
                    nc.vector.tensor_copy(out=aldbuf[1][:, ti], in_=ps4[:, 2:4])
                    nc.vector.tensor_copy(out=rbc[:, t, 3:5], in_=alsb[:])
                nc.sync.dma_start(
                    shard[1][s * SUP1 * P : (s + 1) * SUP1 * P, :]
                    .rearrange("(t p) f -> p t f", p=P), rbc[:])
            tc.strict_bb_all_engine_barrier()
            nc.gpsimd.collective_compute(
                "AllGather", OP.bypass, replica_groups=[core_ids],
                ins=[shard[1][:]], outs=[tabn[1][:]])
            tc.strict_bb_all_engine_barrier()

            for L, F, Hc, HC in LAYERS:
                SUP = SUPS[L]
                K = layer_K[L]
                ns = T // SUP
                tab_ap = tabn[L]
                conv = conv_t[L]

                # ============ edge phase ============
                gofs = 0
                for s in range(ns):
                    Ks = [int(K[s, c]) for c in range(NWIN)]
                    S = sum(Ks)
                    if S == 0:
                        cvz = wp.tile([P, SUP, HC], f32, tag="cvz", name="cvz")
                        nc.vector.memset(cvz[:], 0.0)
                        nc.sync.dma_start(
                            conv[s * SUP : (s + 1) * SUP].rearrange("t p f -> p t f"),
                            cvz[:])
                        continue
                    gsb = iop.tile([P, 8 * SUP * S], i16, tag="gsb", name="gsb")
                    nc.sync.dma_start(gsb[:], gidx_t[L][:, gofs : gofs + 8 * SUP * S])
                    gofs += 8 * SUP * S
                    gt = gpool.tile([P, SUP * S, 128], bf16, tag="gt", name="gt",
                                    bufs=int(os.environ.get("GAT_GTBUFS", "2")))
                    so = 0
                    CAPC = int(os.environ.get('GAT_CAPC', '8'))  # 1024-idx HW limit
                    for c in range(NWIN):
                        k = Ks[c]
                        if k == 0:
                            continue
                        win = tab_ap[c * CH : min(c * CH + CH, NT), :]
                        base = SUP * so
                        tot = SUP * k
                        for ofs in range(0, tot, CAPC):
                            w = min(CAPC, tot - ofs)
                            nidx = w * P
                            nc.gpsimd.dma_gather(
                                gt[:, base + ofs : base + ofs + w, :], win,
                                gsb[:, 8 * (base + ofs) : 8 * (base + ofs + w)],
                                nidx, nidx, 128)
                        so += k

                    ald_ap = aldbuf[L][:, s * SUP : (s + 1) * SUP, :]

                    scr = wp.tile([P, SUP, S, 2], f32, tag="scr", name="scr")
                    so = 0
                    for c in range(NWIN):
                        k = Ks[c]
                        if k == 0:
                            continue
                        in0 = gt[:, SUP * so : SUP * (so + k), F : F + 2]
                        in0 = in0.rearrange("p (t k) h -> p t k h", k=k)
                        in1 = ald_ap.unsqueeze(2).to_broadcast([P, SUP, k, 2])
                        nc.vector.tensor_tensor(
                            out=scr[:, :, so : so + k, :], in0=in0, in1=in1,
                            op=OP.add)
                        so += k
                    ex = wp.tile([P, SUP, S, 2], f32, tag="ex", name="ex")
                    nc.vector.tensor_scalar_mul(out=ex[:], in0=scr[:],
                                                scalar1=NEG_SLOPE_CONST)
                    nc.vector.tensor_tensor(out=ex[:], in0=ex[:], in1=scr[:],
                                            op=OP.max)
                    nc.scalar.activation(out=ex[:], in_=ex[:], func=AF.Exp)
                    den = wp.tile([P, SUP, 2], f32, tag="den", name="den")
                    nc.vector.tensor_reduce(
                        out=den[:], in_=ex[:].rearrange("p t s h -> p t h s"),
                        axis=mybir.AxisListType.X, op=OP.add)
                    nc.vector.tensor_scalar_max(out=den[:], in0=den[:], scalar1=1e-30)
                    rden = wp.tile([P, SUP, 2], f32, tag="rden", name="rden")
                    nc.vector.reciprocal(rden[:], den[:])
                    alph = wp.tile([P, SUP, S, 2], bf16, tag="alph", name="alph")
                    nc.vector.tensor_tensor(
                        out=alph[:], in0=ex[:],
                        in1=rden[:].unsqueeze(2).to_broadcast([P, SUP, S, 2]),
                        op=OP.mult)
                    # tmp layout [P, t, h, s, F]: multiply fully contiguous
                    # (inner f stride 1 on all streams); the single-stream
                    # reduce pays the stride instead.
                    tmp = wp.tile([P, SUP, 2, S, F], bf16, tag="tmp", name="tmp",
                                  bufs=int(os.environ.get("GAT_TMPBUFS", "1")))
                    so = 0
                    for c in range(NWIN):
                        k = Ks[c]
                        if k == 0:
                            continue
                        in0 = gt[:, SUP * so : SUP * (so + k), 0:F]
                        in0 = in0.rearrange("p (t k) f -> p t k f", k=k)
                        for h in range(2):
                            in1 = alph[:, :, so : so + k, h : h + 1]
                            in1 = in1.to_broadcast([P, SUP, k, F])
                            nc.vector.tensor_tensor(
                                out=tmp[:, :, h, so : so + k, :], in0=in0,
                                in1=in1, op=OP.mult)
                        so += k
                    agg = wp.tile([P, SUP, 2, F], f32, tag="agg", name="agg")
                    nc.vector.tensor_reduce(
                        out=agg[:].rearrange("p t h f -> p (t h) f"),
                        in_=tmp[:].rearrange("p t h s f -> p (t h) f s"),
                        axis=mybir.AxisListType.X, op=OP.add)
                    for t in range(SUP):
                        ti = s * SUP + t
                        cvp = psp.tile([P, HC], f32, tag="cvp", name="cvp")
                        for h in range(2):
                            tp = psp.tile([F, P], f32, tag="tp", name="tp")
                            nc.tensor.transpose(
                                out=tp[:], in_=agg[:, t, h, :],
                                identity=ident[:])
                            aggT = wp.tile([F, P], f32, tag="aggT", name="aggT")
                            nc.vector.tensor_copy(out=aggT[:], in_=tp[:])
                            nc.tensor.matmul(
                                out=cvp[:, h * Hc : (h + 1) * Hc],
                                lhsT=aggT[:],
                                rhs=wsb[L][:, h * Hc : (h + 1) * Hc],
                                start=True, stop=True)
                        cvs = wp.tile([P, HC], f32, tag="cvs", name="cvs")
                        nc.vector.tensor_copy(out=cvs[:], in_=cvp[:])
                        nc.sync.dma_start(conv[ti], cvs[:])

                tc.strict_bb_all_engine_barrier()

                # ============ stats ============
                acc = pp.tile([P, 2 * HC], f32, tag=f"acc{L}", name=f"acc{L}")
                nc.vector.memset(acc[:], 0.0)
                TB = SUP
                for b in range(T // TB):
                    cvb = iop.tile([P, TB, HC], f32, tag="cvb", name="cvb")
                    nc.sync.dma_start(
                        cvb[:], conv[b * TB : (b + 1) * TB].rearrange("t p f -> p t f"))
                    mkb = iop.tile([P, TB], f32, tag="mkb", name="mkb")
                    nc.sync.dma_start(
                        mkb[:], nmask_t[b * TB : (b + 1) * TB].rearrange("t p -> p t"))
                    cvm = wp.tile([P, TB, HC], f32, tag="cvm", name="cvm")
                    nc.vector.tensor_tensor(
                        out=cvm[:], in0=cvb[:],
                        in1=mkb[:].unsqueeze(2).to_broadcast([P, TB, HC]), op=OP.mult)
                    sq = wp.tile([P, TB, HC], f32, tag="sqt", name="sqt")
                    nc.vector.tensor_tensor(out=sq[:], in0=cvm[:], in1=cvb[:],
                                            op=OP.mult)
                    r1 = wp.tile([P, HC], f32, tag="r1", name="r1")
                    nc.vector.tensor_reduce(
                        out=r1[:], in_=cvm[:].rearrange("p t f -> p f t"),
                        axis=mybir.AxisListType.X, op=OP.add)
                    nc.vector.tensor_tensor(out=acc[:, 0:HC], in0=acc[:, 0:HC],
                                            in1=r1[:], op=OP.add)
                    r2 = wp.tile([P, HC], f32, tag="r2", name="r2")
                    nc.vector.tensor_reduce(
                        out=r2[:], in_=sq[:].rearrange("p t f -> p f t"),
                        axis=mybir.AxisListType.X, op=OP.add)
                    nc.vector.tensor_tensor(out=acc[:, HC:], in0=acc[:, HC:],
                                            in1=r2[:], op=OP.add)
                stp = psp.tile([1, 2 * HC], f32, tag="stp", name="stp", bufs=1)
                nc.tensor.matmul(out=stp[:], lhsT=ones[:], rhs=acc[:],
                                 start=True, stop=True)
                sts = wp.tile([1, 2 * HC], f32, tag="sts", name="sts")
                nc.vector.tensor_copy(out=sts[:], in_=stp[:])
                nc.sync.dma_start(stin[L][:], sts[:])
                tc.strict_bb_all_engine_barrier()
                nc.gpsimd.collective_compute(
                    "AllReduce", OP.add, replica_groups=[core_ids],
                    ins=[stin[L][:]], outs=[stout[L][:]])
                stg = wp.tile([1, 2 * HC], f32, tag="stg", name="stg")
                nc.sync.dma_start(stg[:], stout[L][:])
                mu = wp.tile([1, HC], f32, tag="mu", name="mu")
                nc.vector.tensor_scalar_mul(out=mu[:], in0=stg[:, 0:HC],
                                            scalar1=1.0 / N)
                var = wp.tile([1, HC], f32, tag="var", name="var")
                nc.vector.tensor_scalar_mul(out=var[:], in0=stg[:, HC:],
                                            scalar1=1.0 / N)
                mu2 = wp.tile([1, HC], f32, tag="mu2", name="mu2")
                nc.vector.tensor_tensor(out=mu2[:], in0=mu[:], in1=mu[:], op=OP.mult)
                nc.vector.tensor_tensor(out=var[:], in0=var[:], in1=mu2[:],
                                        op=OP.subtract)
                nc.vector.tensor_scalar_add(out=var[:], in0=var[:], scalar1=EPS)
                sd = wp.tile([1, HC], f32, tag="sd", name="sd")
                nc.scalar.activation(out=sd[:], in_=var[:], func=AF.Sqrt)
                rsd = wp.tile([1, HC], f32, tag="rsd", name="rsd")
                nc.vector.reciprocal(rsd[:], sd[:])
                acst = wp.tile([1, 2 * HC], f32, tag="acst", name="acst")
                asc = acst[:, 0:HC]
                csc = acst[:, HC:]
                nc.vector.tensor_tensor(out=asc, in0=gamsb[L][:], in1=rsd[:],
                                        op=OP.mult)
                nc.vector.tensor_tensor(out=csc, in0=mu[:], in1=asc, op=OP.mult)
                nc.vector.tensor_tensor(out=csc, in0=betsb[L][:], in1=csc,
                                        op=OP.subtract)
                bcp = psp.tile([P, 2 * HC], f32, tag="bcp", name="bcp", bufs=1)
                nc.tensor.matmul(out=bcp[:], lhsT=onesr[:], rhs=acst[:],
                                 start=True, stop=True)
                bcs = pp.tile([P, 2 * HC], f32, tag=f"bcs{L}", name=f"bcs{L}")
                nc.vector.tensor_copy(out=bcs[:], in_=bcp[:])

                # ============ BN + next-layer table / pooling ============
                SUP2 = SUP
                for s2 in range(T // SUP2):
                    cvb = iop.tile([P, SUP2, HC], f32, tag="cvb2", name="cvb2")
                    nc.sync.dma_start(
                        cvb[:],
                        conv[s2 * SUP2 : (s2 + 1) * SUP2].rearrange("t p f -> p t f"))
                    mkb = iop.tile([P, SUP2], f32, tag="mkb2", name="mkb2")
                    nc.sync.dma_start(
                        mkb[:],
                        nmask_t[s2 * SUP2 : (s2 + 1) * SUP2].rearrange("t p -> p t"))
                    ngb = iop.tile([P, SUP2], f32, tag="ngb2", name="ngb2")
                    nc.sync.dma_start(
                        ngb[:],
                        negb_t[s2 * SUP2 : (s2 + 1) * SUP2].rearrange("t p -> p t"))
                    inp = wp.tile([P, SUP2, HC], f32, tag="inp", name="inp")
                    a_b = bcs[:, 0:HC].unsqueeze(1).to_broadcast([P, SUP2, HC])
                    nc.vector.tensor_tensor(out=inp[:], in0=cvb[:], in1=a_b,
                                            op=OP.mult)
                    c_b = bcs[:, HC:].unsqueeze(1).to_broadcast([P, SUP2, HC])
                    nc.vector.tensor_tensor(out=inp[:], in0=inp[:], in1=c_b,
                                            op=OP.add)
                    nc.vector.tensor_scalar_max(out=inp[:], in0=inp[:], scalar1=0.0)
                    if L < 3:
                        nc.vector.tensor_tensor(
                            out=inp[:], in0=inp[:],
                            in1=mkb[:].unsqueeze(2).to_broadcast([P, SUP2, HC]),
                            op=OP.mult)
                        for t2 in range(SUP2):
                            ti = s2 * SUP2 + t2
                            tp2 = psp.tile([HC, P], f32, tag="tp", name="tp2")
                            nc.tensor.transpose(out=tp2[:], in_=inp[:, t2],
                                                identity=ident[:])
                            inT = wp.tile([HC, P], f32, tag="inT", name="inT")
                            nc.vector.tensor_copy(out=inT[:], in_=tp2[:])
                            ps4 = psp.tile([P, 4], f32, tag="ps4", name="ps4", bufs=1)
                            nc.tensor.matmul(out=ps4[:], lhsT=inT[:],
                                             rhs=avsb[L + 1][:], start=True, stop=True)
                            alsb = wp.tile([P, 2], f32, tag="alsb", name="alsb")
                            nc.vector.scalar_tensor_tensor(
                                out=alsb[:], in0=ps4[:, 0:2],
                                scalar=mkb[:, t2 : t2 + 1],
                                in1=ngb[:, t2 : t2 + 1].to_broadcast([P, 2]),
                                op0=OP.mult, op1=OP.add)
                            nc.vector.tensor_copy(out=aldbuf[L + 1][:, ti],
                                                  in_=ps4[:, 2:4])
                            rb = wp.tile([P, 128], bf16, tag="rb", name="rb")
                            nc.vector.memset(rb[:], 0.0)
                            nc.vector.tensor_copy(out=rb[:, 0:HC], in_=inp[:, t2])
                            nc.vector.tensor_copy(out=rb[:, HC : HC + 2], in_=alsb[:])
                            nc.sync.dma_start(
                                shard[L + 1][ti * P : (ti + 1) * P, :], rb[:])
                    else:
                        for t2 in range(SUP2):
                            ti = s2 * SUP2 + t2
                            h3g = wp.tile([P, HC], f32, tag="h3g", name="h3g")
                            nc.vector.scalar_tensor_tensor(
                                out=h3g[:], in0=inp[:, t2],
                                scalar=mkb[:, t2 : t2 + 1],
                                in1=ngb[:, t2 : t2 + 1].to_broadcast([P, HC]),
                                op0=OP.mult, op1=OP.add)
                            tp3 = psp.tile([HC, P], f32, tag="tp", name="tp3")
                            nc.tensor.transpose(out=tp3[:], in_=h3g[:],
                                                identity=ident[:])
                            nc.vector.tensor_reduce(
                                out=tmax[:, ti : ti + 1], in_=tp3[:],
                                axis=mybir.AxisListType.X, op=OP.max)
                if L < 3:
                    tc.strict_bb_all_engine_barrier()
                    nc.gpsimd.collective_compute(
                        "AllGather", OP.bypass, replica_groups=[core_ids],
                        ins=[shard[L + 1][:]], outs=[tabn[L + 1][:]])
                    tc.strict_bb_all_engine_barrier()

            # ============ pool combine + FC ============
            for g in range(G):
                mt = wp.tile([P, T], f32, tag="mt", name="mt")
                nc.vector.tensor_tensor(
                    out=mt[:], in0=tmax[:],
                    in1=poolbsb[:, g, :], op=OP.add)
                nc.vector.tensor_reduce(out=pool_sb[:, g : g + 1], in_=mt[:],
                                        axis=mybir.AxisListType.X, op=OP.max)
            nc.sync.dma_start(poolin[:], pool_sb[:])
            tc.strict_bb_all_engine_barrier()
            nc.gpsimd.collective_compute(
                "AllReduce", OP.max, replica_groups=[core_ids],
                ins=[poolin[:]], outs=[poolout[:]])
            poolg = wp.tile([P, G], f32, tag="poolg", name="poolg")
            nc.sync.dma_start(poolg[:], poolout[:])
            fcp = psp.tile([G, 10], f32, tag="fcp", name="fcp", bufs=1)
            nc.tensor.matmul(out=fcp[:], lhsT=poolg[:], rhs=fcwsb[:],
                             start=True, stop=True)
            fco = wp.tile([G, 10], f32, tag="fco", name="fco")
            nc.vector.tensor_tensor(out=fco[:], in0=fcp[:],
                                    in1=fcbsb[:], op=OP.add)
            nc.sync.dma_start(out_t[:], fco[:])

    nc.compile()
    return nc


NEG_SLOPE_CONST = 0.2


# ----------------------------------------------------------------------------
# dispatch: cached jit(shard_map) executable with device-resident statics
# ----------------------------------------------------------------------------

class _Exec:
    """Compiled multi-core dispatcher. Static (structure-derived) inputs are
    device-resident jax arrays; per call only dynamic inputs transfer."""

    def __init__(self, nc, static_maps):
        from concourse import bass2jax as b2j
        from jax.sharding import Mesh, PartitionSpec, NamedSharding
        from jax.experimental.shard_map import shard_map

        b2j.install_neuronx_cc_hook()
        self.nc = nc
        partition_name = (nc.partition_id_tensor.name
                          if nc.partition_id_tensor else None)
        in_names, out_names, out_avals, zero_outs = [], [], [], []
        for alloc in nc.m.functions[0].allocations:
            if not isinstance(alloc, mybir.MemoryLocationSet):
                continue
            assert alloc.memorylocations
            name = alloc.memorylocations[0].name
            if alloc.kind == "ExternalInput":
                if name != partition_name:
                    in_names.append(name)
            elif alloc.kind == "ExternalOutput":
                assert alloc.tensor_shape is not None and alloc.dtype is not None
                out_names.append(name)
                shape = tuple(alloc.tensor_shape)
                dtype = mybir.dt.np(alloc.dtype)
                out_avals.append(jax.core.ShapedArray(shape, dtype))
                zero_outs.append(np.zeros((NCORES * shape[0], *shape[1:]), dtype))
        assert nc.dbg_addr is None, "debug build not supported by fast dispatch"
        n_params = len(in_names)
        self.param_names = list(in_names)
        self.out_names = list(out_names)
        self.out_avals = out_avals
        self.zero_shapes = [(z.shape, z.dtype) for z in zero_outs]
        all_names = in_names + out_names
        if partition_name is not None:
            all_names = all_names + [partition_name]

        def _body(*args):
            operands = list(args)
            if partition_name is not None:
                operands.append(b2j.partition_id_tensor())
            outs = b2j._bass_exec_p.bind(
                *operands,
                out_avals=tuple(out_avals),
                in_names=tuple(all_names),
                out_names=tuple(out_names),
                lowering_input_output_aliases=(),
                sim_require_finite=True,
                sim_require_nnan=True,
                nc=nc,
            )
            return tuple(outs)

        devices = jax.devices()[:NCORES]
        assert len(devices) == NCORES, \
            f"need {NCORES} devices, have {len(jax.devices())}"
        mesh = Mesh(np.asarray(devices), ("core",))
        self.mesh = mesh
        n_outs = len(out_names)
        donate = tuple(range(n_params, n_params + n_outs))
        self.fn = jax.jit(
            shard_map(_body, mesh=mesh,
                      in_specs=(PartitionSpec("core"),) * (n_params + n_outs),
                      out_specs=(PartitionSpec("core"),) * n_outs,
                      check_rep=False),
            donate_argnums=donate, keep_unused=True)
        sh = NamedSharding(mesh, PartitionSpec("core"))
        self.static_dev = {}
        for name in STATIC_NAMES:
            cat = np.concatenate(
                [np.asarray(static_maps[c][name]) for c in range(NCORES)], axis=0)
            self.static_dev[name] = jax.device_put(cat, sh)
        jax.block_until_ready(list(self.static_dev.values()))

    def __call__(self, dyn_maps):
        args = []
        for name in self.param_names:
            if name in self.static_dev:
                args.append(self.static_dev[name])
            else:
                args.append(np.concatenate(
                    [np.asarray(dyn_maps[c][name]) for c in range(NCORES)], axis=0))
        zeros = [np.zeros(s, d) for s, d in self.zero_shapes]
        outs = self.fn(*args, *zeros)
        outs = jax.block_until_ready(outs)
        return {name: np.asarray(outs[i]).reshape(NCORES, *self.out_avals[i].shape)
                for i, name in enumerate(self.out_names)}


# ----------------------------------------------------------------------------
# entry point
# ----------------------------------------------------------------------------

_CACHE = {}


def _get_exec(x, edge_index, batch, G, SUPS, CH):
    import zlib
    key = (x.shape, edge_index.shape, G, CH, tuple(sorted(SUPS.items())),
           zlib.crc32(np.ascontiguousarray(edge_index).tobytes()),
           zlib.crc32(np.ascontiguousarray(batch).tobytes()))
    ent = _CACHE.get(key)
    if ent is None:
        cfg, data, aux = preprocess(np.asarray(x), np.asarray(edge_index),
                                    np.asarray(batch), G, SUPS, CH)
        nc = build(cfg)
        ex = _Exec(nc, data)
        ent = (cfg, data, aux, nc, ex)
        _CACHE.clear()
        _CACHE[key] = ent
    return ent


def run_gat(x, edge_index, batch, prm, G, SUPS=None, CH=32768):
    if SUPS is None:
        SUPS = {1: 2, 2: 2, 3: 2}
    x = np.asarray(x)
    edge_index = np.asarray(edge_index)
    batch = np.asarray(batch)
    if os.environ.get("GAT_SIM"):
        cfg, data, aux = preprocess(x, edge_index, batch, G, SUPS, CH)
        nc = build(cfg)
        dyn = host_params(x, prm, cfg, data, aux)
        from concourse.bass_interp import MultiCoreSim
        sim = MultiCoreSim(nc, num_cores=NCORES, require_finite=False,
                           require_nnan=False)
        cores = list(sim.cores.values())
        for j, cs in enumerate(cores):
            for name, val in {**data[j], **dyn[j]}.items():
                cs.tensor(name)[:] = val
        sim.simulate(check_with_hw=False)
        return np.asarray(cores[0].tensor("out"), np.float32)
    cfg, data, aux, nc, ex = _get_exec(x, edge_index, batch, G, SUPS, CH)
    dyn = host_params(x, prm, cfg, data, aux)
    import time as _t
    t0 = _t.time()
    res = ex(dyn)
    run_gat.last_spmd_wall = _t.time() - t0
    run_gat_cached.last_spmd_wall = run_gat.last_spmd_wall
    return np.asarray(res["out"][0], np.float32)


def run_gat_cached(x, edge_index, batch, prm, G, SUPS=None, CH=32768):
    return run_gat(x, edge_index, batch, prm, G, SUPS=SUPS, CH=CH)


def kernel(**inputs):
    x = np.asarray(inputs["x"], np.float32)
    edge_index = np.asarray(inputs["edge_index"], np.int64)
    batch = np.asarray(inputs["batch"], np.int64)
    prm = {k: np.asarray(v, np.float32) for k, v in inputs.items()
           if k not in ("x", "edge_index", "batch")}
    return run_gat_cached(x, edge_index, batch, prm, G=64)


# revision 3
# speedup vs baseline: 111.6283x; 111.6283x over previous
"""3-layer 2-head GAT + BatchNorm/ReLU + per-graph max-pool + FC on 8 trn2 NeuronCores.

Sharding: graph/data-parallel over dst nodes. Host relabels nodes graph-major
(degree-profile sorted within each graph, padded to graph-aligned 128-node
tiles), packs graphs onto cores, and builds per-core ELL gather schedules with
a cross-core-uniform static shape (all per-core variation is data).

Device per layer: bf16 table rows [in_features | al_src] (256B rows, one row
per node id) are gathered per edge-slot with dma_gather (int16 indices =>
32768-row windows); attention scores/softmax and the weighted feature sum run
on the vector/scalar engines along the ELL free axis; the conv matmul
(agg @ W) runs per-tile on the PE after a transpose. BN stats go through a
tiny AllReduce; next-layer features are replicated with an AllGather of bf16
shard rows; the final per-graph max pool uses an AllReduce(max).

Dispatch: a cached jit(shard_map) executable with the large call-invariant
inputs (gather schedules, masks, pool bias) resident on device; per call only
the node features (per-core [T,P,4] shard) and the small weight tensors are
uploaded. The layer-1 feature table is built on device from the x shard
(transpose+matmul for the attention logits, zero-padded bf16 rows) and
replicated with the same AllGather the later layers use.
"""
import os
import numpy as np
import ml_dtypes

import jax
import concourse.bass as bass
import concourse.bacc as bacc
import concourse.mybir as mybir
import concourse.tile as tile
from concourse._compat import cdiv, get_trn_type
from concourse.library_config import mlp

P = 128
NCORES = 8
NEG = -1.0e30
EPS = 1e-5
f32 = mybir.dt.float32
bf16 = mybir.dt.bfloat16
i16 = mybir.dt.int16
AF = mybir.ActivationFunctionType
OP = mybir.AluOpType

# (layer, Fin, C-per-head, HC=2C)
LAYERS = [(1, 3, 16, 32), (2, 32, 32, 64), (3, 64, 64, 128)]

# call-invariant inputs (functions of edge_index/batch/graph structure only):
# uploaded to device once per preprocessing cache entry.
STATIC_NAMES = ("gidx1", "gidx2", "gidx3", "nodemask", "negb", "poolbias",
                "ident")


def wrap_idxs(flat):
    n = len(flat)
    assert n % 16 == 0
    a = flat.astype(np.int16).reshape(n // 16, 16).T
    return np.ascontiguousarray(np.tile(a, (8, 1)))


# ----------------------------------------------------------------------------
# host preprocessing
# ----------------------------------------------------------------------------

def preprocess(x, edge_index, batch, G, SUPS, CH):
    N = x.shape[0]
    src = np.concatenate([edge_index[0], np.arange(N)]).astype(np.int64)
    dst = np.concatenate([edge_index[1], np.arange(N)]).astype(np.int64)
    batch = np.asarray(batch).astype(np.int64)
    counts = np.bincount(batch, minlength=G)
    gstart = np.concatenate([[0], np.cumsum(counts)])

    nw0 = max(cdiv(N, CH), 1)
    prof = np.zeros((N, nw0), np.int64)
    np.add.at(prof, (dst, src // CH), 1)
    order = np.empty(N, np.int64)
    for g in range(G):
        s, e = int(gstart[g]), int(gstart[g + 1])
        idx = np.arange(s, e)
        key = np.lexsort(tuple(prof[s:e, c] for c in range(nw0 - 1, -1, -1)))
        order[s:e] = idx[key]

    tiles = []
    for g in range(G):
        s, e = int(gstart[g]), int(gstart[g + 1])
        ids = order[s:e]
        nt = cdiv(max(len(ids), 1), P)
        pad = np.full(nt * P, -1, np.int64)
        pad[: len(ids)] = ids
        for t in range(nt):
            tiles.append((g, pad[t * P : (t + 1) * P]))

    by_g = {}
    for g, arr in tiles:
        by_g.setdefault(g, []).append(arr)
    core_tiles = [[] for _ in range(NCORES)]
    loads = [0] * NCORES
    for g in sorted(by_g, key=lambda g: -len(by_g[g])):
        j = int(np.argmin(loads))
        for arr in by_g[g]:
            core_tiles[j].append((g, arr))
        loads[j] += len(by_g[g])
    SUPMAX = max(SUPS.values())
    T = cdiv(max(loads), SUPMAX) * SUPMAX
    for j in range(NCORES):
        while len(core_tiles[j]) < T:
            core_tiles[j].append((-1, np.full(P, -1, np.int64)))
    NT = NCORES * T * P
    NWIN = cdiv(NT, CH)

    def assign(ct):
        o2n = np.full(N, -1, np.int64)
        n2o = np.full(NT, -1, np.int64)
        for j in range(NCORES):
            for i, (g, arr) in enumerate(ct[j]):
                base = (j * T + i) * P
                r = arr >= 0
                o2n[arr[r]] = base + np.nonzero(r)[0]
                n2o[base : base + P][r] = arr[r]
        return o2n, n2o

    def k_of(o2n):
        sn, dn = o2n[src], o2n[dst]
        cnt = np.bincount(dn * NWIN + sn // CH, minlength=NT * NWIN).reshape(NT, NWIN)
        return sn, dn, cnt.reshape(NCORES, T, P, NWIN).max(axis=2)

    o2n, n2o = assign(core_tiles)

    # Re-sort nodes within each graph by their FINAL-window in-degree profile
    # (graph->core packing moved graphs to entirely different id windows, so
    # the original-order profile sort is stale). Converges in one iteration:
    # within-graph reordering moves ids by < graph size << window size.
    for _ in range(2):
        prof2 = np.zeros((N, NWIN), np.int64)
        sn = o2n[src]
        np.add.at(prof2, (dst, sn // CH), 1)
        for j in range(NCORES):
            by_graph = {}
            for i, (g, arr) in enumerate(core_tiles[j]):
                by_graph.setdefault(g, []).append(i)
            for g, idxs_t in by_graph.items():
                if g < 0:
                    continue
                ids = np.concatenate([core_tiles[j][i][1] for i in idxs_t])
                ids = ids[ids >= 0]
                key = np.lexsort(tuple(prof2[ids, c]
                                       for c in range(NWIN - 1, -1, -1)))
                ids = ids[key]
                pad = np.full(len(idxs_t) * P, -1, np.int64)
                pad[: len(ids)] = ids
                for n_i, i in enumerate(idxs_t):
                    core_tiles[j][i] = (g, pad[n_i * P : (n_i + 1) * P])
        o2n, n2o = assign(core_tiles)

    _, _, Kc = k_of(o2n)
    for j in range(NCORES):
        key = np.argsort(-Kc[j].sum(axis=1), kind="stable")
        core_tiles[j] = [core_tiles[j][i] for i in key]
    o2n, n2o = assign(core_tiles)
    snew, dnew, Kc = k_of(o2n)
    Kuni = Kc.max(axis=0)  # [T, NWIN]

    layer_K = {}
    for L, SUP in SUPS.items():
        ns = T // SUP
        K = np.zeros((ns, NWIN), np.int64)
        for s in range(ns):
            K[s] = Kuni[s * SUP : (s + 1) * SUP].max(axis=0)
        layer_K[L] = K

    filler = np.nonzero(n2o < 0)[0]
    pad_row = {}
    for c in range(NWIN):
        f = filler[(filler // CH) == c]
        assert len(f) > 0, f"no filler row in window {c}"
        pad_row[c] = int(f[0])

    schunk = snew // CH
    eo = np.lexsort((snew, schunk, dnew))
    ds, cs, ss = dnew[eo], schunk[eo], snew[eo]
    grp = ds * NWIN + cs
    firsts = np.ones(len(grp), bool)
    firsts[1:] = grp[1:] != grp[:-1]
    g0 = np.nonzero(firsts)[0]
    rank = np.arange(len(grp)) - np.repeat(g0, np.diff(np.concatenate([g0, [len(grp)]])))
    KMAX = max(int(Kuni.max()), 1)
    slot = np.full((NT, NWIN, KMAX), -1, np.int64)
    slot[ds, cs, rank] = ss

    data = [dict() for _ in range(NCORES)]
    gcols = {}
    for L, SUP in SUPS.items():
        K = layer_K[L]
        ns = T // SUP
        for j in range(NCORES):
            segs = []
            for s in range(ns):
                for c in range(NWIN):
                    k = int(K[s, c])
                    if k == 0:
                        continue
                    rows = slot[(j * T + s * SUP) * P : (j * T + (s + 1) * SUP) * P, c, :k]
                    sub = rows.reshape(SUP, P, k)
                    loc = np.where(sub < 0, pad_row[c], sub) - c * CH
                    assert loc.min() >= 0 and loc.max() < CH
                    segs.append(wrap_idxs(loc.transpose(0, 2, 1).reshape(-1)))
            arr = (np.concatenate(segs, axis=1) if segs
                   else np.zeros((P, 8), np.int16))
            data[j][f"gidx{L}"] = np.ascontiguousarray(arr)
        gcols[L] = data[0][f"gidx{L}"].shape[1]
        for j in range(NCORES):
            assert data[j][f"gidx{L}"].shape[1] == gcols[L]

    for j in range(NCORES):
        nm = np.zeros((T, P), np.float32)
        pb = np.full((G, T), NEG, np.float32)
        for i, (g, arr) in enumerate(core_tiles[j]):
            nm[i] = (arr >= 0).astype(np.float32)
            if g >= 0:
                pb[g, i] = 0.0
        data[j]["nodemask"] = nm
        data[j]["negb"] = ((nm - 1.0) * 1e30).astype(np.float32)
        data[j]["poolbias"] = np.ascontiguousarray(
            np.repeat(pb[:, None, :], P, axis=1).reshape(G * P, T))
        data[j]["ident"] = np.eye(128, dtype=np.float32)

    cfg = dict(N=int(N), G=int(G), T=int(T), NT=int(NT), NWIN=int(NWIN), CH=int(CH),
               SUPS=dict(SUPS), gcols=gcols,
               layer_K={L: K.astype(int) for L, K in layer_K.items()})
    aux = dict(o2n=o2n, n2o=n2o)
    return cfg, data, aux


def host_params(x, prm, cfg, data, aux):
    """Per-call (x/param-dependent) inputs: per-core x shard + small weights."""
    n2o = aux["n2o"]
    T, NT = cfg["T"], cfg["NT"]

    def avec(W, a_s, a_d, F, Hc):
        Wr = np.asarray(W).reshape(F, 2, Hc)
        vs = np.stack([Wr[:, h, :] @ np.asarray(a_s)[h] for h in range(2)], axis=1)
        vd = np.stack([Wr[:, h, :] @ np.asarray(a_d)[h] for h in range(2)], axis=1)
        return np.concatenate([vs, vd], axis=1).astype(np.float32)

    av1 = avec(prm["W1"], prm["as1"], prm["ad1"], 3, 16)
    av1p = np.zeros((4, 4), np.float32)
    av1p[0:3] = av1
    av2 = avec(prm["W2"], prm["as2"], prm["ad2"], 32, 32)
    av3 = avec(prm["W3"], prm["as3"], prm["ad3"], 64, 64)

    shared = {
        "w1": np.asarray(prm["W1"], np.float32),
        "w2": np.asarray(prm["W2"], np.float32),
        "w3": np.asarray(prm["W3"], np.float32),
        "avec1": av1p, "avec2": av2, "avec3": av3,
        "fcw": np.asarray(prm["fcw"], np.float32),
        "fcb": np.tile(np.asarray(prm["fcb"], np.float32).reshape(1, -1), (cfg["G"], 1)),
    }
    for L, F, Hc, HC in LAYERS:
        shared[f"gam{L}"] = np.asarray(prm[f"g{L}"], np.float32).reshape(1, -1)
        shared[f"bet{L}"] = np.asarray(prm[f"be{L}"], np.float32).reshape(1, -1)
    xr = np.asarray(x, np.float32)
    dyn = [dict(shared) for _ in range(NCORES)]
    for j in range(NCORES):
        ids = n2o[j * T * P : (j + 1) * T * P]
        xs = np.zeros((T * P, 4), np.float32)
        r = ids >= 0
        xs[r, 0:3] = xr[ids[r]]
        dyn[j]["xs"] = xs.reshape(T, P, 4)
    return dyn


# ----------------------------------------------------------------------------
# device program
# ----------------------------------------------------------------------------

def build(cfg):
    T, NT, NWIN, G, N, CH = (cfg["T"], cfg["NT"], cfg["NWIN"], cfg["G"],
                             cfg["N"], cfg["CH"])
    SUPS, layer_K, gcols = cfg["SUPS"], cfg["layer_K"], cfg["gcols"]
    core_ids = list(range(NCORES))

    nc = bacc.Bacc(get_trn_type() or "TRN2", target_bir_lowering=False)

    xs_t = nc.dram_tensor("xs", [T, P, 4], f32, kind="ExternalInput")
    gidx_t = {L: nc.dram_tensor(f"gidx{L}", [P, gcols[L]], i16, kind="ExternalInput")
              for L, _, _, _ in LAYERS}
    nmask_t = nc.dram_tensor("nodemask", [T, P], f32, kind="ExternalInput")
    negb_t = nc.dram_tensor("negb", [T, P], f32, kind="ExternalInput")
    poolb_t = nc.dram_tensor("poolbias", [G * P, T], f32, kind="ExternalInput")
    w_t = {1: nc.dram_tensor("w1", [3, 32], f32, kind="ExternalInput"),
           2: nc.dram_tensor("w2", [32, 64], f32, kind="ExternalInput"),
           3: nc.dram_tensor("w3", [64, 128], f32, kind="ExternalInput")}
    av_t = {1: nc.dram_tensor("avec1", [4, 4], f32, kind="ExternalInput"),
            2: nc.dram_tensor("avec2", [32, 4], f32, kind="ExternalInput"),
            3: nc.dram_tensor("avec3", [64, 4], f32, kind="ExternalInput")}
    gam_t = {L: nc.dram_tensor(f"gam{L}", [1, HC], f32, kind="ExternalInput")
             for L, F, Hc, HC in LAYERS}
    bet_t = {L: nc.dram_tensor(f"bet{L}", [1, HC], f32, kind="ExternalInput")
             for L, F, Hc, HC in LAYERS}
    fcw_t = nc.dram_tensor("fcw", [128, 10], f32, kind="ExternalInput")
    fcb_t = nc.dram_tensor("fcb", [G, 10], f32, kind="ExternalInput")
    ident_t = nc.dram_tensor("ident", [P, P], f32, kind="ExternalInput")
    out_t = nc.dram_tensor("out", [G, 10], f32, kind="ExternalOutput")

    # internal DRAM
    tabn = {1: nc.dram_tensor("tabA", [NT, 128], bf16, addr_space="Shared"),
            2: nc.dram_tensor("tabB", [NT, 128], bf16, addr_space="Shared"),
            3: nc.dram_tensor("tabC", [NT, 128], bf16, addr_space="Shared")}
    shard = {1: nc.dram_tensor("shardA", [T * P, 128], bf16),
             2: nc.dram_tensor("shardB", [T * P, 128], bf16),
             3: nc.dram_tensor("shardC", [T * P, 128], bf16)}
    conv_t = {L: nc.dram_tensor(f"conv{L}", [T, P, HC], f32)
              for L, F, Hc, HC in LAYERS}
    stin = {L: nc.dram_tensor(f"stin{L}", [1, 2 * HC], f32)
            for L, F, Hc, HC in LAYERS}
    stout = {L: nc.dram_tensor(f"stout{L}", [1, 2 * HC], f32, addr_space="Shared")
             for L, F, Hc, HC in LAYERS}
    poolin = nc.dram_tensor("poolin", [P, G], f32)
    poolout = nc.dram_tensor("poolout", [P, G], f32, addr_space="Shared")

    with tile.TileContext(nc) as tc:
        with (
            tc.tile_pool(name="persist", bufs=1) as pp,
            tc.tile_pool(name="io", bufs=int(os.environ.get("GAT_IOBUFS", "3"))) as iop,
            tc.tile_pool(name="gath", bufs=2) as gpool,
            tc.tile_pool(name="work", bufs=int(os.environ.get("GAT_WPBUFS", "2"))) as wp,
            tc.tile_pool(name="psum", bufs=2, space="PSUM") as psp,
        ):
            nc.gpsimd.load_library(mlp)
            tc.strict_bb_all_engine_barrier()
            ident = pp.tile([P, P], f32, tag="ident", name="ident")
            nc.sync.dma_start(ident[:], ident_t[:])
            ones = pp.tile([P, 1], f32, tag="ones", name="ones")
            nc.vector.memset(ones[:], 1.0)
            onesr = pp.tile([1, P], f32, tag="onesr", name="onesr")
            nc.vector.memset(onesr[:], 1.0)

            wsb = {}
            for L, F, Hc, HC in LAYERS:
                wsb[L] = pp.tile([F, HC], f32, tag=f"w{L}", name=f"w{L}")
                nc.sync.dma_start(wsb[L][:], w_t[L][:])
            avsb = {}
            for L in (1, 2, 3):
                Fin = 4 if L == 1 else LAYERS[L - 1][1]
                avsb[L] = pp.tile([Fin, 4], f32, tag=f"av{L}", name=f"av{L}")
                nc.sync.dma_start(avsb[L][:], av_t[L][:])
            gamsb, betsb = {}, {}
            for L, F, Hc, HC in LAYERS:
                gamsb[L] = pp.tile([1, HC], f32, tag=f"gam{L}", name=f"gam{L}")
                betsb[L] = pp.tile([1, HC], f32, tag=f"bet{L}", name=f"bet{L}")
                nc.sync.dma_start(gamsb[L][:], gam_t[L][:])
                nc.sync.dma_start(betsb[L][:], bet_t[L][:])
            fcwsb = pp.tile([128, 10], f32, tag="fcw", name="fcw")
            nc.sync.dma_start(fcwsb[:], fcw_t[:])
            fcbsb = pp.tile([G, 10], f32, tag="fcb", name="fcb")
            nc.sync.dma_start(fcbsb[:], fcb_t[:])
            poolbsb = pp.tile([P, G, T], f32, tag="poolb", name="poolb")
            nc.sync.dma_start(poolbsb[:], poolb_t[:].rearrange("(g p) t -> p g t", p=P))
            aldbuf = {L: pp.tile([P, T, 2], f32, tag=f"ald{L}", name=f"ald{L}")
                      for L in (1, 2, 3)}
            tmax = pp.tile([P, T], f32, tag="tmax", name="tmax")
            pool_sb = pp.tile([P, G], f32, tag="pool", name="pool")

            # ============ layer-1 table build (shard rows from x) ============
            SUP1 = SUPS[1]
            for s in range(T // SUP1):
                xst = iop.tile([P, SUP1, 4], f32, tag="xst", name="xst")
                nc.sync.dma_start(
                    xst[:], xs_t[s * SUP1 : (s + 1) * SUP1].rearrange("t p f -> p t f"))
                mkb = iop.tile([P, SUP1], f32, tag="mkb0", name="mkb0")
                nc.sync.dma_start(
                    mkb[:], nmask_t[s * SUP1 : (s + 1) * SUP1].rearrange("t p -> p t"))
                ngb = iop.tile([P, SUP1], f32, tag="ngb0", name="ngb0")
                nc.sync.dma_start(
                    ngb[:], negb_t[s * SUP1 : (s + 1) * SUP1].rearrange("t p -> p t"))
                rbc = wp.tile([P, SUP1, 128], bf16, tag="rbc", name="rbc")
                nc.vector.memset(rbc[:], 0.0)
                nc.vector.tensor_copy(out=rbc[:, :, 0:3], in_=xst[:, :, 0:3])
                for t in range(SUP1):
                    ti = s * SUP1 + t
                    tp0 = psp.tile([4, P], f32, tag="tp", name="tp0")
                    nc.tensor.transpose(out=tp0[:], in_=xst[:, t, :],
                                        identity=ident[:])
                    xT = wp.tile([4, P], f32, tag="xT", name="xT")
                    nc.vector.tensor_copy(out=xT[:], in_=tp0[:])
                    ps4 = psp.tile([P, 4], f32, tag="ps4", name="ps40", bufs=1)
                    nc.tensor.matmul(out=ps4[:], lhsT=xT[:], rhs=avsb[1][:],
                                     start=True, stop=True)
                    alsb = wp.tile([P, 2], f32, tag="alsb0", name="alsb0")
                    nc.vector.scalar_tensor_tensor(
                        out=alsb[:], in0=ps4[:, 0:2],
                        scalar=mkb[:, t : t + 1],
                        in1=ngb[:, t : t + 1].to_broadcast([P, 2]),
                        op0=OP.mult, op1=OP.add)
                    nc.vector.tensor_copy(out=aldbuf[1][:, ti], in_=ps4[:, 2:4])
                    nc.vector.tensor_copy(out=rbc[:, t, 3:5], in_=alsb[:])
                nc.sync.dma_start(
                    shard[1][s * SUP1 * P : (s + 1) * SUP1 * P, :]
                    .rearrange("(t p) f -> p t f", p=P), rbc[:])
            tc.strict_bb_all_engine_barrier()
            nc.gpsimd.collective_compute(
                "AllGather", OP.bypass, replica_groups=[core_ids],
                ins=[shard[1][:]], outs=[tabn[1][:]])
            tc.strict_bb_all_engine_barrier()

            for L, F, Hc, HC in LAYERS:
                SUP = SUPS[L]
                K = layer_K[L]
                ns = T // SUP
                tab_ap = tabn[L]
                conv = conv_t[L]

                # ============ edge phase ============
                gofs = 0
                for s in range(ns):
                    Ks = [int(K[s, c]) for c in range(NWIN)]
                    S = sum(Ks)
                    if S == 0:
                        cvz = wp.tile([P, SUP, HC], f32, tag="cvz", name="cvz")
                        nc.vector.memset(cvz[:], 0.0)
                        nc.sync.dma_start(
                            conv[s * SUP : (s + 1) * SUP].rearrange("t p f -> p t f"),
                            cvz[:])
                        continue
                    gsb = iop.tile([P, 8 * SUP * S], i16, tag="gsb", name="gsb")
                    nc.sync.dma_start(gsb[:], gidx_t[L][:, gofs : gofs + 8 * SUP * S])
                    gofs += 8 * SUP * S
                    gt = gpool.tile([P, SUP * S, 128], bf16, tag="gt", name="gt",
                                    bufs=int(os.environ.get("GAT_GTBUFS", "2")))
                    so = 0
                    CAPC = int(os.environ.get('GAT_CAPC', '8'))  # 1024-idx HW limit
                    for c in range(NWIN):
                        k = Ks[c]
                        if k == 0:
                            continue
                        win = tab_ap[c * CH : min(c * CH + CH, NT), :]
                        base = SUP * so
                        tot = SUP * k
                        for ofs in range(0, tot, CAPC):
                            w = min(CAPC, tot - ofs)
                            nidx = w * P
                            nc.gpsimd.dma_gather(
                                gt[:, base + ofs : base + ofs + w, :], win,
                                gsb[:, 8 * (base + ofs) : 8 * (base + ofs + w)],
                                nidx, nidx, 128)
                        so += k

                    ald_ap = aldbuf[L][:, s * SUP : (s + 1) * SUP, :]

                    scr = wp.tile([P, SUP, S, 2], f32, tag="scr", name="scr")
                    so = 0
                    for c in range(NWIN):
                        k = Ks[c]
                        if k == 0:
                            continue
                        in0 = gt[:, SUP * so : SUP * (so + k), F : F + 2]
                        in0 = in0.rearrange("p (t k) h -> p t k h", k=k)
                        in1 = ald_ap.unsqueeze(2).to_broadcast([P, SUP, k, 2])
                        nc.vector.tensor_tensor(
                            out=scr[:, :, so : so + k, :], in0=in0, in1=in1,
                            op=OP.add)
                        so += k
                    ex = wp.tile([P, SUP, S, 2], f32, tag="ex", name="ex")
                    nc.vector.tensor_scalar_mul(out=ex[:], in0=scr[:],
                                                scalar1=NEG_SLOPE_CONST)
                    nc.vector.tensor_tensor(out=ex[:], in0=ex[:], in1=scr[:],
                                            op=OP.max)
                    nc.scalar.activation(out=ex[:], in_=ex[:], func=AF.Exp)
                    den = wp.tile([P, SUP, 2], f32, tag="den", name="den")
                    nc.vector.tensor_reduce(
                        out=den[:], in_=ex[:].rearrange("p t s h -> p t h s"),
                        axis=mybir.AxisListType.X, op=OP.add)
                    nc.vector.tensor_scalar_max(out=den[:], in0=den[:], scalar1=1e-30)
                    rden = wp.tile([P, SUP, 2], f32, tag="rden", name="rden")
                    nc.vector.reciprocal(rden[:], den[:])
                    alph = wp.tile([P, SUP, S, 2], bf16, tag="alph", name="alph")
                    nc.vector.tensor_tensor(
                        out=alph[:], in0=ex[:],
                        in1=rden[:].unsqueeze(2).to_broadcast([P, SUP, S, 2]),
                        op=OP.mult)
                    # tmp layout [P, t, h, s, F]: multiply fully contiguous
                    # (inner f stride 1 on all streams); the single-stream
                    # reduce pays the stride instead.
                    tmp = wp.tile([P, SUP, 2, S, F], bf16, tag="tmp", name="tmp",
                                  bufs=int(os.environ.get("GAT_TMPBUFS", "1")))
                    so = 0
                    for c in range(NWIN):
                        k = Ks[c]
                        if k == 0:
                            continue
                        in0 = gt[:, SUP * so : SUP * (so + k), 0:F]
                        in0 = in0.rearrange("p (t k) f -> p t k f", k=k)
                        for h in range(2):
                            in1 = alph[:, :, so : so + k, h : h + 1]
                            in1 = in1.to_broadcast([P, SUP, k, F])
                            nc.vector.tensor_tensor(
                                out=tmp[:, :, h, so : so + k, :], in0=in0,
                                in1=in1, op=OP.mult)
                        so += k
                    agg = wp.tile([P, SUP, 2, F], f32, tag="agg", name="agg")
                    nc.vector.tensor_reduce(
                        out=agg[:].rearrange("p t h f -> p (t h) f"),
                        in_=tmp[:].rearrange("p t h s f -> p (t h) f s"),
                        axis=mybir.AxisListType.X, op=OP.add)
                    for t in range(SUP):
                        ti = s * SUP + t
                        cvp = psp.tile([P, HC], f32, tag="cvp", name="cvp")
                        for h in range(2):
                            tp = psp.tile([F, P], f32, tag="tp", name="tp")
                            nc.tensor.transpose(
                                out=tp[:], in_=agg[:, t, h, :],
                                identity=ident[:])
                            aggT = wp.tile([F, P], f32, tag="aggT", name="aggT")
                            nc.vector.tensor_copy(out=aggT[:], in_=tp[:])
                            nc.tensor.matmul(
                                out=cvp[:, h * Hc : (h + 1) * Hc],
                                lhsT=aggT[:],
                                rhs=wsb[L][:, h * Hc : (h + 1) * Hc],
                                start=True, stop=True)
                        cvs = wp.tile([P, HC], f32, tag="cvs", name="cvs")
                        nc.vector.tensor_copy(out=cvs[:], in_=cvp[:])
                        nc.sync.dma_start(conv[ti], cvs[:])

                tc.strict_bb_all_engine_barrier()

                # ============ stats ============
                acc = pp.tile([P, 2 * HC], f32, tag=f"acc{L}", name=f"acc{L}")
                nc.vector.memset(acc[:], 0.0)
                TB = SUP
                for b in range(T // TB):
                    cvb = iop.tile([P, TB, HC], f32, tag="cvb", name="cvb")
                    nc.sync.dma_start(
                        cvb[:], conv[b * TB : (b + 1) * TB].rearrange("t p f -> p t f"))
                    mkb = iop.tile([P, TB], f32, tag="mkb", name="mkb")
                    nc.sync.dma_start(
                        mkb[:], nmask_t[b * TB : (b + 1) * TB].rearrange("t p -> p t"))
                    cvm = wp.tile([P, TB, HC], f32, tag="cvm", name="cvm")
                    nc.vector.tensor_tensor(
                        out=cvm[:], in0=cvb[:],
                        in1=mkb[:].unsqueeze(2).to_broadcast([P, TB, HC]), op=OP.mult)
                    sq = wp.tile([P, TB, HC], f32, tag="sqt", name="sqt")
                    nc.vector.tensor_tensor(out=sq[:], in0=cvm[:], in1=cvb[:],
                                            op=OP.mult)
                    r1 = wp.tile([P, HC], f32, tag="r1", name="r1")
                    nc.vector.tensor_reduce(
                        out=r1[:], in_=cvm[:].rearrange("p t f -> p f t"),
                        axis=mybir.AxisListType.X, op=OP.add)
                    nc.vector.tensor_tensor(out=acc[:, 0:HC], in0=acc[:, 0:HC],
                                            in1=r1[:], op=OP.add)
                    r2 = wp.tile([P, HC], f32, tag="r2", name="r2")
                    nc.vector.tensor_reduce(
                        out=r2[:], in_=sq[:].rearrange("p t f -> p f t"),
                        axis=mybir.AxisListType.X, op=OP.add)
                    nc.vector.tensor_tensor(out=acc[:, HC:], in0=acc[:, HC:],
                                            in1=r2[:], op=OP.add)
                stp = psp.tile([1, 2 * HC], f32, tag="stp", name="stp", bufs=1)
                nc.tensor.matmul(out=stp[:], lhsT=ones[:], rhs=acc[:],
                                 start=True, stop=True)
                sts = wp.tile([1, 2 * HC], f32, tag="sts", name="sts")
                nc.vector.tensor_copy(out=sts[:], in_=stp[:])
                nc.sync.dma_start(stin[L][:], sts[:])
                tc.strict_bb_all_engine_barrier()
                nc.gpsimd.collective_compute(
                    "AllReduce", OP.add, replica_groups=[core_ids],
                    ins=[stin[L][:]], outs=[stout[L][:]])
                stg = wp.tile([1, 2 * HC], f32, tag="stg", name="stg")
                nc.sync.dma_start(stg[:], stout[L][:])
                mu = wp.tile([1, HC], f32, tag="mu", name="mu")
                nc.vector.tensor_scalar_mul(out=mu[:], in0=stg[:, 0:HC],
                                            scalar1=1.0 / N)
                var = wp.tile([1, HC], f32, tag="var", name="var")
                nc.vector.tensor_scalar_mul(out=var[:], in0=stg[:, HC:],
                                            scalar1=1.0 / N)
                mu2 = wp.tile([1, HC], f32, tag="mu2", name="mu2")
                nc.vector.tensor_tensor(out=mu2[:], in0=mu[:], in1=mu[:], op=OP.mult)
                nc.vector.tensor_tensor(out=var[:], in0=var[:], in1=mu2[:],
                                        op=OP.subtract)
                nc.vector.tensor_scalar_add(out=var[:], in0=var[:], scalar1=EPS)
                sd = wp.tile([1, HC], f32, tag="sd", name="sd")
                nc.scalar.activation(out=sd[:], in_=var[:], func=AF.Sqrt)
                rsd = wp.tile([1, HC], f32, tag="rsd", name="rsd")
                nc.vector.reciprocal(rsd[:], sd[:])
                acst = wp.tile([1, 2 * HC], f32, tag="acst", name="acst")
                asc = acst[:, 0:HC]
                csc = acst[:, HC:]
                nc.vector.tensor_tensor(out=asc, in0=gamsb[L][:], in1=rsd[:],
                                        op=OP.mult)
                nc.vector.tensor_tensor(out=csc, in0=mu[:], in1=asc, op=OP.mult)
                nc.vector.tensor_tensor(out=csc, in0=betsb[L][:], in1=csc,
                                        op=OP.subtract)
                bcp = psp.tile([P, 2 * HC], f32, tag="bcp", name="bcp", bufs=1)
                nc.tensor.matmul(out=bcp[:], lhsT=onesr[:], rhs=acst[:],
                                 start=True, stop=True)
                bcs = pp.tile([P, 2 * HC], f32, tag=f"bcs{L}", name=f"bcs{L}")
                nc.vector.tensor_copy(out=bcs[:], in_=bcp[:])

                # ============ BN + next-layer table / pooling ============
                SUP2 = SUP
                for s2 in range(T // SUP2):
                    cvb = iop.tile([P, SUP2, HC], f32, tag="cvb2", name="cvb2")
                    nc.sync.dma_start(
                        cvb[:],
                        conv[s2 * SUP2 : (s2 + 1) * SUP2].rearrange("t p f -> p t f"))
                    mkb = iop.tile([P, SUP2], f32, tag="mkb2", name="mkb2")
                    nc.sync.dma_start(
                        mkb[:],
                        nmask_t[s2 * SUP2 : (s2 + 1) * SUP2].rearrange("t p -> p t"))
                    ngb = iop.tile([P, SUP2], f32, tag="ngb2", name="ngb2")
                    nc.sync.dma_start(
                        ngb[:],
                        negb_t[s2 * SUP2 : (s2 + 1) * SUP2].rearrange("t p -> p t"))
                    inp = wp.tile([P, SUP2, HC], f32, tag="inp", name="inp")
                    a_b = bcs[:, 0:HC].unsqueeze(1).to_broadcast([P, SUP2, HC])
                    nc.vector.tensor_tensor(out=inp[:], in0=cvb[:], in1=a_b,
                                            op=OP.mult)
                    c_b = bcs[:, HC:].unsqueeze(1).to_broadcast([P, SUP2, HC])
                    nc.vector.tensor_tensor(out=inp[:], in0=inp[:], in1=c_b,
                                            op=OP.add)
                    nc.vector.tensor_scalar_max(out=inp[:], in0=inp[:], scalar1=0.0)
                    if L < 3:
                        nc.vector.tensor_tensor(
                            out=inp[:], in0=inp[:],
                            in1=mkb[:].unsqueeze(2).to_broadcast([P, SUP2, HC]),
                            op=OP.mult)
                        for t2 in range(SUP2):
                            ti = s2 * SUP2 + t2
                            tp2 = psp.tile([HC, P], f32, tag="tp", name="tp2")
                            nc.tensor.transpose(out=tp2[:], in_=inp[:, t2],
                                                identity=ident[:])
                            inT = wp.tile([HC, P], f32, tag="inT", name="inT")
                            nc.vector.tensor_copy(out=inT[:], in_=tp2[:])
                            ps4 = psp.tile([P, 4], f32, tag="ps4", name="ps4", bufs=1)
                            nc.tensor.matmul(out=ps4[:], lhsT=inT[:],
                                             rhs=avsb[L + 1][:], start=True, stop=True)
                            alsb = wp.tile([P, 2], f32, tag="alsb", name="alsb")
                            nc.vector.scalar_tensor_tensor(
                                out=alsb[:], in0=ps4[:, 0:2],
                                scalar=mkb[:, t2 : t2 + 1],
                                in1=ngb[:, t2 : t2 + 1].to_broadcast([P, 2]),
                                op0=OP.mult, op1=OP.add)
                            nc.vector.tensor_copy(out=aldbuf[L + 1][:, ti],
                                                  in_=ps4[:, 2:4])
                            rb = wp.tile([P, 128], bf16, tag="rb", name="rb")
                            nc.vector.memset(rb[:], 0.0)
                            nc.vector.tensor_copy(out=rb[:, 0:HC], in_=inp[:, t2])
                            nc.vector.tensor_copy(out=rb[:, HC : HC + 2], in_=alsb[:])
                            nc.sync.dma_start(
                                shard[L + 1][ti * P : (ti + 1) * P, :], rb[:])
                    else:
                        for t2 in range(SUP2):
                            ti = s2 * SUP2 + t2
                            h3g = wp.tile([P, HC], f32, tag="h3g", name="h3g")
                            nc.vector.scalar_tensor_tensor(
                                out=h3g[:], in0=inp[:, t2],
                                scalar=mkb[:, t2 : t2 + 1],
                                in1=ngb[:, t2 : t2 + 1].to_broadcast([P, HC]),
                                op0=OP.mult, op1=OP.add)
                            tp3 = psp.tile([HC, P], f32, tag="tp", name="tp3")
                            nc.tensor.transpose(out=tp3[:], in_=h3g[:],
                                                identity=ident[:])
                            nc.vector.tensor_reduce(
                                out=tmax[:, ti : ti + 1], in_=tp3[:],
                                axis=mybir.AxisListType.X, op=OP.max)
                if L < 3:
                    tc.strict_bb_all_engine_barrier()
                    nc.gpsimd.collective_compute(
                        "AllGather", OP.bypass, replica_groups=[core_ids],
                        ins=[shard[L + 1][:]], outs=[tabn[L + 1][:]])
                    tc.strict_bb_all_engine_barrier()

            # ============ pool combine + FC ============
            for g in range(G):
                mt = wp.tile([P, T], f32, tag="mt", name="mt")
                nc.vector.tensor_tensor(
                    out=mt[:], in0=tmax[:],
                    in1=poolbsb[:, g, :], op=OP.add)
                nc.vector.tensor_reduce(out=pool_sb[:, g : g + 1], in_=mt[:],
                                        axis=mybir.AxisListType.X, op=OP.max)
            nc.sync.dma_start(poolin[:], pool_sb[:])
            tc.strict_bb_all_engine_barrier()
            nc.gpsimd.collective_compute(
                "AllReduce", OP.max, replica_groups=[core_ids],
                ins=[poolin[:]], outs=[poolout[:]])
            poolg = wp.tile([P, G], f32, tag="poolg", name="poolg")
            nc.sync.dma_start(poolg[:], poolout[:])
            fcp = psp.tile([G, 10], f32, tag="fcp", name="fcp", bufs=1)
            nc.tensor.matmul(out=fcp[:], lhsT=poolg[:], rhs=fcwsb[:],
                             start=True, stop=True)
            fco = wp.tile([G, 10], f32, tag="fco", name="fco")
            nc.vector.tensor_tensor(out=fco[:], in0=fcp[:],
                                    in1=fcbsb[:], op=OP.add)
            nc.sync.dma_start(out_t[:], fco[:])

    nc.compile()
    return nc


NEG_SLOPE_CONST = 0.2


# ----------------------------------------------------------------------------
# dispatch: cached jit(shard_map) executable with device-resident statics
# ----------------------------------------------------------------------------

class _Exec:
    """Compiled multi-core dispatcher. Static (structure-derived) inputs are
    device-resident jax arrays; per call only dynamic inputs transfer."""

    def __init__(self, nc, static_maps):
        from concourse import bass2jax as b2j
        from jax.sharding import Mesh, PartitionSpec, NamedSharding
        from jax.experimental.shard_map import shard_map

        b2j.install_neuronx_cc_hook()
        self.nc = nc
        partition_name = (nc.partition_id_tensor.name
                          if nc.partition_id_tensor else None)
        in_names, out_names, out_avals, zero_outs = [], [], [], []
        for alloc in nc.m.functions[0].allocations:
            if not isinstance(alloc, mybir.MemoryLocationSet):
                continue
            assert alloc.memorylocations
            name = alloc.memorylocations[0].name
            if alloc.kind == "ExternalInput":
                if name != partition_name:
                    in_names.append(name)
            elif alloc.kind == "ExternalOutput":
                assert alloc.tensor_shape is not None and alloc.dtype is not None
                out_names.append(name)
                shape = tuple(alloc.tensor_shape)
                dtype = mybir.dt.np(alloc.dtype)
                out_avals.append(jax.core.ShapedArray(shape, dtype))
                zero_outs.append(np.zeros((NCORES * shape[0], *shape[1:]), dtype))
        assert nc.dbg_addr is None, "debug build not supported by fast dispatch"
        n_params = len(in_names)
        self.param_names = list(in_names)
        self.out_names = list(out_names)
        self.out_avals = out_avals
        self.zero_shapes = [(z.shape, z.dtype) for z in zero_outs]
        all_names = in_names + out_names
        if partition_name is not None:
            all_names = all_names + [partition_name]

        def _body(*args):
            operands = list(args)
            if partition_name is not None:
                operands.append(b2j.partition_id_tensor())
            outs = b2j._bass_exec_p.bind(
                *operands,
                out_avals=tuple(out_avals),
                in_names=tuple(all_names),
                out_names=tuple(out_names),
                lowering_input_output_aliases=(),
                sim_require_finite=True,
                sim_require_nnan=True,
                nc=nc,
            )
            return tuple(outs)

        devices = jax.devices()[:NCORES]
        assert len(devices) == NCORES, \
            f"need {NCORES} devices, have {len(jax.devices())}"
        mesh = Mesh(np.asarray(devices), ("core",))
        self.mesh = mesh
        n_outs = len(out_names)
        donate = tuple(range(n_params, n_params + n_outs))
        self.fn = jax.jit(
            shard_map(_body, mesh=mesh,
                      in_specs=(PartitionSpec("core"),) * (n_params + n_outs),
                      out_specs=(PartitionSpec("core"),) * n_outs,
                      check_rep=False),
            donate_argnums=donate, keep_unused=True)
        sh = NamedSharding(mesh, PartitionSpec("core"))
        self.static_dev = {}
        for name in STATIC_NAMES:
            cat = np.concatenate(
                [np.asarray(static_maps[c][name]) for c in range(NCORES)], axis=0)
            self.static_dev[name] = jax.device_put(cat, sh)
        jax.block_until_ready(list(self.static_dev.values()))

    def __call__(self, dyn_maps):
        args = []
        for name in self.param_names:
            if name in self.static_dev:
                args.append(self.static_dev[name])
            else:
                args.append(np.concatenate(
                    [np.asarray(dyn_maps[c][name]) for c in range(NCORES)], axis=0))
        zeros = [np.zeros(s, d) for s, d in self.zero_shapes]
        outs = self.fn(*args, *zeros)
        outs = jax.block_until_ready(outs)
        return {name: np.asarray(outs[i]).reshape(NCORES, *self.out_avals[i].shape)
                for i, name in enumerate(self.out_names)}


# ----------------------------------------------------------------------------
# entry point
# ----------------------------------------------------------------------------

_CACHE = {}


def _get_exec(x, edge_index, batch, G, SUPS, CH):
    import zlib
    key = (x.shape, edge_index.shape, G, CH, tuple(sorted(SUPS.items())),
           zlib.crc32(np.ascontiguousarray(edge_index).tobytes()),
           zlib.crc32(np.ascontiguousarray(batch).tobytes()))
    ent = _CACHE.get(key)
    if ent is None:
        cfg, data, aux = preprocess(np.asarray(x), np.asarray(edge_index),
                                    np.asarray(batch), G, SUPS, CH)
        nc = build(cfg)
        ex = _Exec(nc, data)
        ent = (cfg, data, aux, nc, ex)
        _CACHE.clear()
        _CACHE[key] = ent
    return ent


def run_gat(x, edge_index, batch, prm, G, SUPS=None, CH=32768):
    if SUPS is None:
        SUPS = {1: 2, 2: 2, 3: 2}
    x = np.asarray(x)
    edge_index = np.asarray(edge_index)
    batch = np.asarray(batch)
    if os.environ.get("GAT_SIM"):
        cfg, data, aux = preprocess(x, edge_index, batch, G, SUPS, CH)
        nc = build(cfg)
        dyn = host_params(x, prm, cfg, data, aux)
        from concourse.bass_interp import MultiCoreSim
        sim = MultiCoreSim(nc, num_cores=NCORES, require_finite=False,
                           require_nnan=False)
        cores = list(sim.cores.values())
        for j, cs in enumerate(cores):
            for name, val in {**data[j], **dyn[j]}.items():
                cs.tensor(name)[:] = val
        sim.simulate(check_with_hw=False)
        return np.asarray(cores[0].tensor("out"), np.float32)
    cfg, data, aux, nc, ex = _get_exec(x, edge_index, batch, G, SUPS, CH)
    dyn = host_params(x, prm, cfg, data, aux)
    import time as _t
    t0 = _t.time()
    res = ex(dyn)
    run_gat.last_spmd_wall = _t.time() - t0
    run_gat_cached.last_spmd_wall = run_gat.last_spmd_wall
    return np.asarray(res["out"][0], np.float32)


def run_gat_cached(x, edge_index, batch, prm, G, SUPS=None, CH=32768):
    return run_gat(x, edge_index, batch, prm, G, SUPS=SUPS, CH=CH)


def kernel(**inputs):
    x = np.asarray(inputs["x"], np.float32)
    edge_index = np.asarray(inputs["edge_index"], np.int64)
    batch = np.asarray(inputs["batch"], np.int64)
    prm = {k: np.asarray(v, np.float32) for k, v in inputs.items()
           if k not in ("x", "edge_index", "batch")}
    return run_gat_cached(x, edge_index, batch, prm, G=64)


# revision 10
# speedup vs baseline: 214.9527x; 1.9256x over previous
"""3-layer 2-head GAT + BatchNorm/ReLU + per-graph max-pool + FC on 8 trn2 NeuronCores.

Sharding: graph/data-parallel over dst nodes. Host relabels nodes graph-major
(degree-profile sorted within each graph, padded to graph-aligned 128-node
tiles), packs graphs onto cores, and builds per-core ELL gather schedules with
a cross-core-uniform static shape (all per-core variation is data).

Device per layer: bf16 table rows [in_features | al_src] (256B rows, one row
per node id) are gathered per edge-slot with dma_gather (int16 indices =>
32768-row windows); attention scores/softmax and the weighted feature sum run
on the vector/scalar engines along the ELL free axis; the conv matmul
(agg @ W) runs per-tile on the PE after a transpose. BN stats go through a
tiny AllReduce; next-layer features are replicated with an AllGather of bf16
shard rows; the final per-graph max pool uses an AllReduce(max).

Dispatch: a cached jit(shard_map) executable with the large call-invariant
inputs (gather schedules, masks, pool bias) resident on device; per call only
the node features (per-core [T,P,4] shard) and the small weight tensors are
uploaded. The layer-1 feature table is built on device from the x shard
(transpose+matmul for the attention logits, zero-padded bf16 rows) and
replicated with the same AllGather the later layers use.
"""
import os
import numpy as np
import ml_dtypes

import jax
import concourse.bass as bass
import concourse.bacc as bacc
import concourse.mybir as mybir
import concourse.tile as tile
from concourse._compat import cdiv, get_trn_type
from concourse.library_config import mlp

P = 128
NCORES = 8
NEG = -1.0e30
EPS = 1e-5
f32 = mybir.dt.float32
bf16 = mybir.dt.bfloat16
i16 = mybir.dt.int16
AF = mybir.ActivationFunctionType
OP = mybir.AluOpType

# (layer, Fin, C-per-head, HC=2C)
LAYERS = [(1, 3, 16, 32), (2, 32, 32, 64), (3, 64, 64, 128)]

# call-invariant inputs (functions of edge_index/batch/graph structure only):
# uploaded to device once per preprocessing cache entry.
STATIC_NAMES = ("gidx1", "gidx2", "gidx3", "nodemask", "negb", "poolbias",
                "ident")


def param_layout(G):
    """(name, shape) of every small weight tensor packed into the pblob, in
    blob order. Shared by build() and host_params()."""
    return [("w1", (3, 32)), ("w2", (32, 64)), ("w3", (64, 128)),
            ("avec1", (4, 4)), ("avec2", (32, 4)), ("avec3", (64, 4)),
            ("gam1", (1, 32)), ("bet1", (1, 32)),
            ("gam2", (1, 64)), ("bet2", (1, 64)),
            ("gam3", (1, 128)), ("bet3", (1, 128)),
            ("fcw", (128, 10)), ("fcb", (G, 10))]


def wrap_idxs(flat):
    n = len(flat)
    assert n % 16 == 0
    a = flat.astype(np.int16).reshape(n // 16, 16).T
    return np.ascontiguousarray(np.tile(a, (8, 1)))


# ----------------------------------------------------------------------------
# host preprocessing
# ----------------------------------------------------------------------------

def preprocess(x, edge_index, batch, G, SUPS, CH):
    N = x.shape[0]
    src = np.concatenate([edge_index[0], np.arange(N)]).astype(np.int64)
    dst = np.concatenate([edge_index[1], np.arange(N)]).astype(np.int64)
    batch = np.asarray(batch).astype(np.int64)
    counts = np.bincount(batch, minlength=G)
    gstart = np.concatenate([[0], np.cumsum(counts)])

    nw0 = max(cdiv(N, CH), 1)
    prof = np.zeros((N, nw0), np.int64)
    np.add.at(prof, (dst, src // CH), 1)
    order = np.empty(N, np.int64)
    for g in range(G):
        s, e = int(gstart[g]), int(gstart[g + 1])
        idx = np.arange(s, e)
        key = np.lexsort(tuple(prof[s:e, c] for c in range(nw0 - 1, -1, -1)))
        order[s:e] = idx[key]

    tiles = []
    for g in range(G):
        s, e = int(gstart[g]), int(gstart[g + 1])
        ids = order[s:e]
        nt = cdiv(max(len(ids), 1), P)
        pad = np.full(nt * P, -1, np.int64)
        pad[: len(ids)] = ids
        for t in range(nt):
            tiles.append((g, pad[t * P : (t + 1) * P]))

    by_g = {}
    for g, arr in tiles:
        by_g.setdefault(g, []).append(arr)
    core_tiles = [[] for _ in range(NCORES)]
    loads = [0] * NCORES
    for g in sorted(by_g, key=lambda g: -len(by_g[g])):
        j = int(np.argmin(loads))
        for arr in by_g[g]:
            core_tiles[j].append((g, arr))
        loads[j] += len(by_g[g])
    SUPMAX = max(SUPS.values())
    T = cdiv(max(loads), SUPMAX) * SUPMAX
    for j in range(NCORES):
        while len(core_tiles[j]) < T:
            core_tiles[j].append((-1, np.full(P, -1, np.int64)))
    NT = NCORES * T * P
    NWIN = cdiv(NT, CH)

    def assign(ct):
        o2n = np.full(N, -1, np.int64)
        n2o = np.full(NT, -1, np.int64)
        for j in range(NCORES):
            for i, (g, arr) in enumerate(ct[j]):
                base = (j * T + i) * P
                r = arr >= 0
                o2n[arr[r]] = base + np.nonzero(r)[0]
                n2o[base : base + P][r] = arr[r]
        return o2n, n2o

    def k_of(o2n):
        sn, dn = o2n[src], o2n[dst]
        cnt = np.bincount(dn * NWIN + sn // CH, minlength=NT * NWIN).reshape(NT, NWIN)
        return sn, dn, cnt.reshape(NCORES, T, P, NWIN).max(axis=2)

    o2n, n2o = assign(core_tiles)

    # Re-sort nodes within each graph by their FINAL-window in-degree profile
    # (graph->core packing moved graphs to entirely different id windows, so
    # the original-order profile sort is stale). Converges in one iteration:
    # within-graph reordering moves ids by < graph size << window size.
    for _ in range(2):
        prof2 = np.zeros((N, NWIN), np.int64)
        sn = o2n[src]
        np.add.at(prof2, (dst, sn // CH), 1)
        for j in range(NCORES):
            by_graph = {}
            for i, (g, arr) in enumerate(core_tiles[j]):
                by_graph.setdefault(g, []).append(i)
            for g, idxs_t in by_graph.items():
                if g < 0:
                    continue
                ids = np.concatenate([core_tiles[j][i][1] for i in idxs_t])
                ids = ids[ids >= 0]
                key = np.lexsort(tuple(prof2[ids, c]
                                       for c in range(NWIN - 1, -1, -1)))
                ids = ids[key]
                pad = np.full(len(idxs_t) * P, -1, np.int64)
                pad[: len(ids)] = ids
                for n_i, i in enumerate(idxs_t):
                    core_tiles[j][i] = (g, pad[n_i * P : (n_i + 1) * P])
        o2n, n2o = assign(core_tiles)

    _, _, Kc = k_of(o2n)
    for j in range(NCORES):
        key = np.argsort(-Kc[j].sum(axis=1), kind="stable")
        core_tiles[j] = [core_tiles[j][i] for i in key]
    o2n, n2o = assign(core_tiles)
    snew, dnew, Kc = k_of(o2n)
    Kuni = Kc.max(axis=0)  # [T, NWIN]

    layer_K = {}
    for L, SUP in SUPS.items():
        ns = T // SUP
        K = np.zeros((ns, NWIN), np.int64)
        for s in range(ns):
            K[s] = Kuni[s * SUP : (s + 1) * SUP].max(axis=0)
        layer_K[L] = K

    filler = np.nonzero(n2o < 0)[0]
    pad_row = {}
    for c in range(NWIN):
        f = filler[(filler // CH) == c]
        assert len(f) > 0, f"no filler row in window {c}"
        pad_row[c] = int(f[0])

    schunk = snew // CH
    eo = np.lexsort((snew, schunk, dnew))
    ds, cs, ss = dnew[eo], schunk[eo], snew[eo]
    grp = ds * NWIN + cs
    firsts = np.ones(len(grp), bool)
    firsts[1:] = grp[1:] != grp[:-1]
    g0 = np.nonzero(firsts)[0]
    rank = np.arange(len(grp)) - np.repeat(g0, np.diff(np.concatenate([g0, [len(grp)]])))
    KMAX = max(int(Kuni.max()), 1)
    slot = np.full((NT, NWIN, KMAX), -1, np.int64)
    slot[ds, cs, rank] = ss

    data = [dict() for _ in range(NCORES)]
    gcols = {}
    for L, SUP in SUPS.items():
        K = layer_K[L]
        ns = T // SUP
        for j in range(NCORES):
            segs = []
            for s in range(ns):
                for c in range(NWIN):
                    k = int(K[s, c])
                    if k == 0:
                        continue
                    rows = slot[(j * T + s * SUP) * P : (j * T + (s + 1) * SUP) * P, c, :k]
                    sub = rows.reshape(SUP, P, k)
                    loc = np.where(sub < 0, pad_row[c], sub) - c * CH
                    assert loc.min() >= 0 and loc.max() < CH
                    segs.append(wrap_idxs(loc.transpose(0, 2, 1).reshape(-1)))
            arr = (np.concatenate(segs, axis=1) if segs
                   else np.zeros((P, 8), np.int16))
            data[j][f"gidx{L}"] = np.ascontiguousarray(arr)
        gcols[L] = data[0][f"gidx{L}"].shape[1]
        for j in range(NCORES):
            assert data[j][f"gidx{L}"].shape[1] == gcols[L]

    for j in range(NCORES):
        nm = np.zeros((T, P), np.float32)
        pb = np.full((G, T), NEG, np.float32)
        for i, (g, arr) in enumerate(core_tiles[j]):
            nm[i] = (arr >= 0).astype(np.float32)
            if g >= 0:
                pb[g, i] = 0.0
        data[j]["nodemask"] = nm
        data[j]["negb"] = ((nm - 1.0) * 1e30).astype(np.float32)
        data[j]["poolbias"] = np.ascontiguousarray(
            np.repeat(pb[:, None, :], P, axis=1).reshape(G * P, T))
        data[j]["ident"] = np.eye(128, dtype=np.float32)

    cfg = dict(N=int(N), G=int(G), T=int(T), NT=int(NT), NWIN=int(NWIN), CH=int(CH),
               SUPS=dict(SUPS), gcols=gcols,
               layer_K={L: K.astype(int) for L, K in layer_K.items()})
    aux = dict(o2n=o2n, n2o=n2o)
    return cfg, data, aux


def host_params(x, prm, cfg, data, aux):
    """Per-call (x/param-dependent) inputs: per-core x shard + small weights."""
    n2o = aux["n2o"]
    T, NT = cfg["T"], cfg["NT"]

    def avec(W, a_s, a_d, F, Hc):
        Wr = np.asarray(W).reshape(F, 2, Hc)
        vs = np.stack([Wr[:, h, :] @ np.asarray(a_s)[h] for h in range(2)], axis=1)
        vd = np.stack([Wr[:, h, :] @ np.asarray(a_d)[h] for h in range(2)], axis=1)
        return np.concatenate([vs, vd], axis=1).astype(np.float32)

    av1 = avec(prm["W1"], prm["as1"], prm["ad1"], 3, 16)
    av1p = np.zeros((4, 4), np.float32)
    av1p[0:3] = av1
    av2 = avec(prm["W2"], prm["as2"], prm["ad2"], 32, 32)
    av3 = avec(prm["W3"], prm["as3"], prm["ad3"], 64, 64)

    vals = {
        "w1": np.asarray(prm["W1"], np.float32),
        "w2": np.asarray(prm["W2"], np.float32),
        "w3": np.asarray(prm["W3"], np.float32),
        "avec1": av1p, "avec2": av2, "avec3": av3,
        "fcw": np.asarray(prm["fcw"], np.float32),
        "fcb": np.tile(np.asarray(prm["fcb"], np.float32).reshape(1, -1), (cfg["G"], 1)),
    }
    for L, F, Hc, HC in LAYERS:
        vals[f"gam{L}"] = np.asarray(prm[f"g{L}"], np.float32).reshape(1, -1)
        vals[f"bet{L}"] = np.asarray(prm[f"be{L}"], np.float32).reshape(1, -1)
    parts = []
    for name, shape in param_layout(cfg["G"]):
        v = vals[name]
        assert v.shape == shape, (name, v.shape, shape)
        parts.append(v.ravel())
    pblob = np.ascontiguousarray(np.concatenate(parts).astype(np.float32))
    xr = np.asarray(x, np.float32)
    dyn = [{"pblob": pblob} for _ in range(NCORES)]
    for j in range(NCORES):
        ids = n2o[j * T * P : (j + 1) * T * P]
        xs = np.zeros((T * P, 4), np.float32)
        r = ids >= 0
        xs[r, 0:3] = xr[ids[r]]
        dyn[j]["xs"] = xs.reshape(T, P, 4).astype(ml_dtypes.bfloat16)
    return dyn


# ----------------------------------------------------------------------------
# device program
# ----------------------------------------------------------------------------

def build(cfg):
    T, NT, NWIN, G, N, CH = (cfg["T"], cfg["NT"], cfg["NWIN"], cfg["G"],
                             cfg["N"], cfg["CH"])
    SUPS, layer_K, gcols = cfg["SUPS"], cfg["layer_K"], cfg["gcols"]
    core_ids = list(range(NCORES))

    nc = bacc.Bacc(get_trn_type() or "TRN2", target_bir_lowering=False)

    xs_t = nc.dram_tensor("xs", [T, P, 4], bf16, kind="ExternalInput")
    gidx_t = {L: nc.dram_tensor(f"gidx{L}", [P, gcols[L]], i16, kind="ExternalInput")
              for L, _, _, _ in LAYERS}
    nmask_t = nc.dram_tensor("nodemask", [T, P], f32, kind="ExternalInput")
    negb_t = nc.dram_tensor("negb", [T, P], f32, kind="ExternalInput")
    poolb_t = nc.dram_tensor("poolbias", [G * P, T], f32, kind="ExternalInput")
    playout = param_layout(G)
    NPRM = sum(a * b for _, (a, b) in playout)
    pblob_t = nc.dram_tensor("pblob", [1, NPRM], f32, kind="ExternalInput")
    pview = {}
    off = 0
    for name, (a, b) in playout:
        pview[name] = pblob_t[:, off : off + a * b].rearrange(
            "o (a b) -> (o a) b", a=a)
        off += a * b
    ident_t = nc.dram_tensor("ident", [P, P], f32, kind="ExternalInput")
    out_t = nc.dram_tensor("out", [G, 10], f32, kind="ExternalOutput")

    # internal DRAM
    tabn = {1: nc.dram_tensor("tabA", [NT, 128], bf16, addr_space="Shared"),
            2: nc.dram_tensor("tabB", [NT, 128], bf16, addr_space="Shared"),
            3: nc.dram_tensor("tabC", [NT, 128], bf16, addr_space="Shared")}
    shard = {1: nc.dram_tensor("shardA", [T * P, 128], bf16),
             2: nc.dram_tensor("shardB", [T * P, 128], bf16),
             3: nc.dram_tensor("shardC", [T * P, 128], bf16)}
    conv_t = {L: nc.dram_tensor(f"conv{L}", [T, P, HC], f32)
              for L, F, Hc, HC in LAYERS}
    stin = {L: nc.dram_tensor(f"stin{L}", [1, 2 * HC], f32)
            for L, F, Hc, HC in LAYERS}
    stout = {L: nc.dram_tensor(f"stout{L}", [1, 2 * HC], f32, addr_space="Shared")
             for L, F, Hc, HC in LAYERS}
    poolin = nc.dram_tensor("poolin", [P, G], f32)
    poolout = nc.dram_tensor("poolout", [P, G], f32, addr_space="Shared")

    with tile.TileContext(nc) as tc:
        with (
            tc.tile_pool(name="persist", bufs=1) as pp,
            tc.tile_pool(name="io", bufs=int(os.environ.get("GAT_IOBUFS", "3"))) as iop,
            tc.tile_pool(name="gath", bufs=2) as gpool,
            tc.tile_pool(name="work", bufs=int(os.environ.get("GAT_WPBUFS", "2"))) as wp,
            tc.tile_pool(name="psum", bufs=2, space="PSUM") as psp,
        ):
            nc.gpsimd.load_library(mlp)
            tc.strict_bb_all_engine_barrier()
            ident = pp.tile([P, P], f32, tag="ident", name="ident")
            nc.sync.dma_start(ident[:], ident_t[:])
            ones = pp.tile([P, 1], f32, tag="ones", name="ones")
            nc.vector.memset(ones[:], 1.0)
            onesr = pp.tile([1, P], f32, tag="onesr", name="onesr")
            nc.vector.memset(onesr[:], 1.0)

            wsb = {}
            for L, F, Hc, HC in LAYERS:
                wsb[L] = pp.tile([F, HC], f32, tag=f"w{L}", name=f"w{L}")
                nc.sync.dma_start(wsb[L][:], pview[f"w{L}"])
            avsb = {}
            for L in (1, 2, 3):
                Fin = 4 if L == 1 else LAYERS[L - 1][1]
                avsb[L] = pp.tile([Fin, 4], f32, tag=f"av{L}", name=f"av{L}")
                nc.sync.dma_start(avsb[L][:], pview[f"avec{L}"])
            gamsb, betsb = {}, {}
            for L, F, Hc, HC in LAYERS:
                gamsb[L] = pp.tile([1, HC], f32, tag=f"gam{L}", name=f"gam{L}")
                betsb[L] = pp.tile([1, HC], f32, tag=f"bet{L}", name=f"bet{L}")
                nc.sync.dma_start(gamsb[L][:], pview[f"gam{L}"])
                nc.sync.dma_start(betsb[L][:], pview[f"bet{L}"])
            fcwsb = pp.tile([128, 10], f32, tag="fcw", name="fcw")
            nc.sync.dma_start(fcwsb[:], pview["fcw"])
            fcbsb = pp.tile([G, 10], f32, tag="fcb", name="fcb")
            nc.sync.dma_start(fcbsb[:], pview["fcb"])
            poolbsb = pp.tile([P, G, T], f32, tag="poolb", name="poolb")
            nc.sync.dma_start(poolbsb[:], poolb_t[:].rearrange("(g p) t -> p g t", p=P))
            aldbuf = {L: pp.tile([P, T, 2], f32, tag=f"ald{L}", name=f"ald{L}")
                      for L in (1, 2, 3)}
            tmax = pp.tile([P, T], f32, tag="tmax", name="tmax")
            pool_sb = pp.tile([P, G], f32, tag="pool", name="pool")

            # ============ layer-1 table build (shard rows from x) ============
            SUP1 = SUPS[1]
            for s in range(T // SUP1):
                xst = iop.tile([P, SUP1, 4], bf16, tag="xst", name="xst")
                nc.sync.dma_start(
                    xst[:], xs_t[s * SUP1 : (s + 1) * SUP1].rearrange("t p f -> p t f"))
                mkb = iop.tile([P, SUP1], f32, tag="mkb0", name="mkb0")
                nc.sync.dma_start(
                    mkb[:], nmask_t[s * SUP1 : (s + 1) * SUP1].rearrange("t p -> p t"))
                ngb = iop.tile([P, SUP1], f32, tag="ngb0", name="ngb0")
                nc.sync.dma_start(
                    ngb[:], negb_t[s * SUP1 : (s + 1) * SUP1].rearrange("t p -> p t"))
                rbc = wp.tile([P, SUP1, 128], bf16, tag="rbc", name="rbc")
                nc.vector.memset(rbc[:], 0.0)
                nc.vector.tensor_copy(out=rbc[:, :, 0:3], in_=xst[:, :, 0:3])
                xstf = wp.tile([P, SUP1, 4], f32, tag="xstf", name="xstf")
                nc.vector.tensor_copy(out=xstf[:], in_=xst[:])
                for t in range(SUP1):
                    ti = s * SUP1 + t
                    tp0 = psp.tile([4, P], f32, tag="tp", name="tp0")
                    nc.tensor.transpose(out=tp0[:], in_=xstf[:, t, :],
                                        identity=ident[:])
                    xT = wp.tile([4, P], f32, tag="xT", name="xT")
                    nc.vector.tensor_copy(out=xT[:], in_=tp0[:])
                    ps4 = psp.tile([P, 4], f32, tag="ps4", name="ps40", bufs=1)
                    nc.tensor.matmul(out=ps4[:], lhsT=xT[:], rhs=avsb[1][:],
                                     start=True, stop=True)
                    alsb = wp.tile([P, 2], f32, tag="alsb0", name="alsb0")
                    nc.vector.scalar_tensor_tensor(
                        out=alsb[:], in0=ps4[:, 0:2],
                        scalar=mkb[:, t : t + 1],
                        in1=ngb[:, t : t + 1].to_broadcast([P, 2]),
                        op0=OP.mult, op1=OP.add)
                    nc.vector.tensor_copy(out=aldbuf[1][:, ti], in_=ps4[:, 2:4])
                    nc.vector.tensor_copy(out=rbc[:, t, 3:5], in_=alsb[:])
                nc.sync.dma_start(
                    shard[1][s * SUP1 * P : (s + 1) * SUP1 * P, :]
                    .rearrange("(t p) f -> p t f", p=P), rbc[:])
            tc.strict_bb_all_engine_barrier()
            nc.gpsimd.collective_compute(
                "AllGather", OP.bypass, replica_groups=[core_ids],
                ins=[shard[1][:]], outs=[tabn[1][:]])
            tc.strict_bb_all_engine_barrier()

            for L, F, Hc, HC in LAYERS:
                SUP = SUPS[L]
                K = layer_K[L]
                ns = T // SUP
                tab_ap = tabn[L]
                conv = conv_t[L]

                # ============ edge phase ============
                gofs = 0
                for s in range(ns):
                    Ks = [int(K[s, c]) for c in range(NWIN)]
                    S = sum(Ks)
                    if S == 0:
                        cvz = wp.tile([P, SUP, HC], f32, tag="cvz", name="cvz")
                        nc.vector.memset(cvz[:], 0.0)
                        nc.sync.dma_start(
                            conv[s * SUP : (s + 1) * SUP].rearrange("t p f -> p t f"),
                            cvz[:])
                        continue
                    gsb = iop.tile([P, 8 * SUP * S], i16, tag="gsb", name="gsb")
                    nc.sync.dma_start(gsb[:], gidx_t[L][:, gofs : gofs + 8 * SUP * S])
                    gofs += 8 * SUP * S
                    gt = gpool.tile([P, SUP * S, 128], bf16, tag="gt", name="gt",
                                    bufs=int(os.environ.get("GAT_GTBUFS", "2")))
                    so = 0
                    CAPC = int(os.environ.get('GAT_CAPC', '8'))  # 1024-idx HW limit
                    for c in range(NWIN):
                        k = Ks[c]
                        if k == 0:
                            continue
                        win = tab_ap[c * CH : min(c * CH + CH, NT), :]
                        base = SUP * so
                        tot = SUP * k
                        for ofs in range(0, tot, CAPC):
                            w = min(CAPC, tot - ofs)
                            nidx = w * P
                            nc.gpsimd.dma_gather(
                                gt[:, base + ofs : base + ofs + w, :], win,
                                gsb[:, 8 * (base + ofs) : 8 * (base + ofs + w)],
                                nidx, nidx, 128)
                        so += k

                    ald_ap = aldbuf[L][:, s * SUP : (s + 1) * SUP, :]

                    scr = wp.tile([P, SUP, S, 2], f32, tag="scr", name="scr")
                    so = 0
                    for c in range(NWIN):
                        k = Ks[c]
                        if k == 0:
                            continue
                        in0 = gt[:, SUP * so : SUP * (so + k), F : F + 2]
                        in0 = in0.rearrange("p (t k) h -> p t k h", k=k)
                        in1 = ald_ap.unsqueeze(2).to_broadcast([P, SUP, k, 2])
                        nc.vector.tensor_tensor(
                            out=scr[:, :, so : so + k, :], in0=in0, in1=in1,
                            op=OP.add)
                        so += k
                    ex = wp.tile([P, SUP, S, 2], f32, tag="ex", name="ex")
                    nc.vector.tensor_scalar_mul(out=ex[:], in0=scr[:],
                                                scalar1=NEG_SLOPE_CONST)
                    nc.vector.tensor_tensor(out=ex[:], in0=ex[:], in1=scr[:],
                                            op=OP.max)
                    nc.scalar.activation(out=ex[:], in_=ex[:], func=AF.Exp)
                    den = wp.tile([P, SUP, 2], f32, tag="den", name="den")
                    nc.vector.tensor_reduce(
                        out=den[:], in_=ex[:].rearrange("p t s h -> p t h s"),
                        axis=mybir.AxisListType.X, op=OP.add)
                    nc.vector.tensor_scalar_max(out=den[:], in0=den[:], scalar1=1e-30)
                    rden = wp.tile([P, SUP, 2], f32, tag="rden", name="rden")
                    nc.vector.reciprocal(rden[:], den[:])
                    alph = wp.tile([P, SUP, S, 2], bf16, tag="alph", name="alph")
                    nc.vector.tensor_tensor(
                        out=alph[:], in0=ex[:],
                        in1=rden[:].unsqueeze(2).to_broadcast([P, SUP, S, 2]),
                        op=OP.mult)
                    # tmp layout [P, t, h, s, F]: multiply fully contiguous
                    # (inner f stride 1 on all streams); the single-stream
                    # reduce pays the stride instead.
                    tmp = wp.tile([P, SUP, 2, S, F], bf16, tag="tmp", name="tmp",
                                  bufs=int(os.environ.get("GAT_TMPBUFS", "1")))
                    so = 0
                    for c in range(NWIN):
                        k = Ks[c]
                        if k == 0:
                            continue
                        in0 = gt[:, SUP * so : SUP * (so + k), 0:F]
                        in0 = in0.rearrange("p (t k) f -> p t k f", k=k)
                        for h in range(2):
                            in1 = alph[:, :, so : so + k, h : h + 1]
                            in1 = in1.to_broadcast([P, SUP, k, F])
                            nc.vector.tensor_tensor(
                                out=tmp[:, :, h, so : so + k, :], in0=in0,
                                in1=in1, op=OP.mult)
                        so += k
                    agg = wp.tile([P, SUP, 2, F], f32, tag="agg", name="agg")
                    nc.vector.tensor_reduce(
                        out=agg[:].rearrange("p t h f -> p (t h) f"),
                        in_=tmp[:].rearrange("p t h s f -> p (t h) f s"),
                        axis=mybir.AxisListType.X, op=OP.add)
                    for t in range(SUP):
                        ti = s * SUP + t
                        cvp = psp.tile([P, HC], f32, tag="cvp", name="cvp")
                        for h in range(2):
                            tp = psp.tile([F, P], f32, tag="tp", name="tp")
                            nc.tensor.transpose(
                                out=tp[:], in_=agg[:, t, h, :],
                                identity=ident[:])
                            aggT = wp.tile([F, P], f32, tag="aggT", name="aggT")
                            nc.vector.tensor_copy(out=aggT[:], in_=tp[:])
                            nc.tensor.matmul(
                                out=cvp[:, h * Hc : (h + 1) * Hc],
                                lhsT=aggT[:],
                                rhs=wsb[L][:, h * Hc : (h + 1) * Hc],
                                start=True, stop=True)
                        cvs = wp.tile([P, HC], f32, tag="cvs", name="cvs")
                        nc.vector.tensor_copy(out=cvs[:], in_=cvp[:])
                        nc.sync.dma_start(conv[ti], cvs[:])

                tc.strict_bb_all_engine_barrier()

                # ============ stats ============
                acc = pp.tile([P, 2 * HC], f32, tag=f"acc{L}", name=f"acc{L}")
                nc.vector.memset(acc[:], 0.0)
                TB = SUP
                for b in range(T // TB):
                    cvb = iop.tile([P, TB, HC], f32, tag="cvb", name="cvb")
                    nc.sync.dma_start(
                        cvb[:], conv[b * TB : (b + 1) * TB].rearrange("t p f -> p t f"))
                    mkb = iop.tile([P, TB], f32, tag="mkb", name="mkb")
                    nc.sync.dma_start(
                        mkb[:], nmask_t[b * TB : (b + 1) * TB].rearrange("t p -> p t"))
                    cvm = wp.tile([P, TB, HC], f32, tag="cvm", name="cvm")
                    nc.vector.tensor_tensor(
                        out=cvm[:], in0=cvb[:],
                        in1=mkb[:].unsqueeze(2).to_broadcast([P, TB, HC]), op=OP.mult)
                    sq = wp.tile([P, TB, HC], f32, tag="sqt", name="sqt")
                    nc.vector.tensor_tensor(out=sq[:], in0=cvm[:], in1=cvb[:],
                                            op=OP.mult)
                    r1 = wp.tile([P, HC], f32, tag="r1", name="r1")
                    nc.vector.tensor_reduce(
                        out=r1[:], in_=cvm[:].rearrange("p t f -> p f t"),
                        axis=mybir.AxisListType.X, op=OP.add)
                    nc.vector.tensor_tensor(out=acc[:, 0:HC], in0=acc[:, 0:HC],
                                            in1=r1[:], op=OP.add)
                    r2 = wp.tile([P, HC], f32, tag="r2", name="r2")
                    nc.vector.tensor_reduce(
                        out=r2[:], in_=sq[:].rearrange("p t f -> p f t"),
                        axis=mybir.AxisListType.X, op=OP.add)
                    nc.vector.tensor_tensor(out=acc[:, HC:], in0=acc[:, HC:],
                                            in1=r2[:], op=OP.add)
                stp = psp.tile([1, 2 * HC], f32, tag="stp", name="stp", bufs=1)
                nc.tensor.matmul(out=stp[:], lhsT=ones[:], rhs=acc[:],
                                 start=True, stop=True)
                sts = wp.tile([1, 2 * HC], f32, tag="sts", name="sts")
                nc.vector.tensor_copy(out=sts[:], in_=stp[:])
                nc.sync.dma_start(stin[L][:], sts[:])
                tc.strict_bb_all_engine_barrier()
                nc.gpsimd.collective_compute(
                    "AllReduce", OP.add, replica_groups=[core_ids],
                    ins=[stin[L][:]], outs=[stout[L][:]])
                stg = wp.tile([1, 2 * HC], f32, tag="stg", name="stg")
                nc.sync.dma_start(stg[:], stout[L][:])
                mu = wp.tile([1, HC], f32, tag="mu", name="mu")
                nc.vector.tensor_scalar_mul(out=mu[:], in0=stg[:, 0:HC],
                                            scalar1=1.0 / N)
                var = wp.tile([1, HC], f32, tag="var", name="var")
                nc.vector.tensor_scalar_mul(out=var[:], in0=stg[:, HC:],
                                            scalar1=1.0 / N)
                mu2 = wp.tile([1, HC], f32, tag="mu2", name="mu2")
                nc.vector.tensor_tensor(out=mu2[:], in0=mu[:], in1=mu[:], op=OP.mult)
                nc.vector.tensor_tensor(out=var[:], in0=var[:], in1=mu2[:],
                                        op=OP.subtract)
                nc.vector.tensor_scalar_add(out=var[:], in0=var[:], scalar1=EPS)
                sd = wp.tile([1, HC], f32, tag="sd", name="sd")
                nc.scalar.activation(out=sd[:], in_=var[:], func=AF.Sqrt)
                rsd = wp.tile([1, HC], f32, tag="rsd", name="rsd")
                nc.vector.reciprocal(rsd[:], sd[:])
                acst = wp.tile([1, 2 * HC], f32, tag="acst", name="acst")
                asc = acst[:, 0:HC]
                csc = acst[:, HC:]
                nc.vector.tensor_tensor(out=asc, in0=gamsb[L][:], in1=rsd[:],
                                        op=OP.mult)
                nc.vector.tensor_tensor(out=csc, in0=mu[:], in1=asc, op=OP.mult)
                nc.vector.tensor_tensor(out=csc, in0=betsb[L][:], in1=csc,
                                        op=OP.subtract)
                bcp = psp.tile([P, 2 * HC], f32, tag="bcp", name="bcp", bufs=1)
                nc.tensor.matmul(out=bcp[:], lhsT=onesr[:], rhs=acst[:],
                                 start=True, stop=True)
                bcs = pp.tile([P, 2 * HC], f32, tag=f"bcs{L}", name=f"bcs{L}")
                nc.vector.tensor_copy(out=bcs[:], in_=bcp[:])

                # ============ BN + next-layer table / pooling ============
                SUP2 = SUP
                for s2 in range(T // SUP2):
                    cvb = iop.tile([P, SUP2, HC], f32, tag="cvb2", name="cvb2")
                    nc.sync.dma_start(
                        cvb[:],
                        conv[s2 * SUP2 : (s2 + 1) * SUP2].rearrange("t p f -> p t f"))
                    mkb = iop.tile([P, SUP2], f32, tag="mkb2", name="mkb2")
                    nc.sync.dma_start(
                        mkb[:],
                        nmask_t[s2 * SUP2 : (s2 + 1) * SUP2].rearrange("t p -> p t"))
                    ngb = iop.tile([P, SUP2], f32, tag="ngb2", name="ngb2")
                    nc.sync.dma_start(
                        ngb[:],
                        negb_t[s2 * SUP2 : (s2 + 1) * SUP2].rearrange("t p -> p t"))
                    inp = wp.tile([P, SUP2, HC], f32, tag="inp", name="inp")
                    a_b = bcs[:, 0:HC].unsqueeze(1).to_broadcast([P, SUP2, HC])
                    nc.vector.tensor_tensor(out=inp[:], in0=cvb[:], in1=a_b,
                                            op=OP.mult)
                    c_b = bcs[:, HC:].unsqueeze(1).to_broadcast([P, SUP2, HC])
                    nc.vector.tensor_tensor(out=inp[:], in0=inp[:], in1=c_b,
                                            op=OP.add)
                    nc.vector.tensor_scalar_max(out=inp[:], in0=inp[:], scalar1=0.0)
                    if L < 3:
                        nc.vector.tensor_tensor(
                            out=inp[:], in0=inp[:],
                            in1=mkb[:].unsqueeze(2).to_broadcast([P, SUP2, HC]),
                            op=OP.mult)
                        for t2 in range(SUP2):
                            ti = s2 * SUP2 + t2
                            tp2 = psp.tile([HC, P], f32, tag="tp", name="tp2")
                            nc.tensor.transpose(out=tp2[:], in_=inp[:, t2],
                                                identity=ident[:])
                            inT = wp.tile([HC, P], f32, tag="inT", name="inT")
                            nc.vector.tensor_copy(out=inT[:], in_=tp2[:])
                            ps4 = psp.tile([P, 4], f32, tag="ps4", name="ps4", bufs=1)
                            nc.tensor.matmul(out=ps4[:], lhsT=inT[:],
                                             rhs=avsb[L + 1][:], start=True, stop=True)
                            alsb = wp.tile([P, 2], f32, tag="alsb", name="alsb")
                            nc.vector.scalar_tensor_tensor(
                                out=alsb[:], in0=ps4[:, 0:2],
                                scalar=mkb[:, t2 : t2 + 1],
                                in1=ngb[:, t2 : t2 + 1].to_broadcast([P, 2]),
                                op0=OP.mult, op1=OP.add)
                            nc.vector.tensor_copy(out=aldbuf[L + 1][:, ti],
                                                  in_=ps4[:, 2:4])
                            rb = wp.tile([P, 128], bf16, tag="rb", name="rb")
                            nc.vector.memset(rb[:], 0.0)
                            nc.vector.tensor_copy(out=rb[:, 0:HC], in_=inp[:, t2])
                            nc.vector.tensor_copy(out=rb[:, HC : HC + 2], in_=alsb[:])
                            nc.sync.dma_start(
                                shard[L + 1][ti * P : (ti + 1) * P, :], rb[:])
                    else:
                        for t2 in range(SUP2):
                            ti = s2 * SUP2 + t2
                            h3g = wp.tile([P, HC], f32, tag="h3g", name="h3g")
                            nc.vector.scalar_tensor_tensor(
                                out=h3g[:], in0=inp[:, t2],
                                scalar=mkb[:, t2 : t2 + 1],
                                in1=ngb[:, t2 : t2 + 1].to_broadcast([P, HC]),
                                op0=OP.mult, op1=OP.add)
                            tp3 = psp.tile([HC, P], f32, tag="tp", name="tp3")
                            nc.tensor.transpose(out=tp3[:], in_=h3g[:],
                                                identity=ident[:])
                            nc.vector.tensor_reduce(
                                out=tmax[:, ti : ti + 1], in_=tp3[:],
                                axis=mybir.AxisListType.X, op=OP.max)
                if L < 3:
                    tc.strict_bb_all_engine_barrier()
                    nc.gpsimd.collective_compute(
                        "AllGather", OP.bypass, replica_groups=[core_ids],
                        ins=[shard[L + 1][:]], outs=[tabn[L + 1][:]])
                    tc.strict_bb_all_engine_barrier()

            # ============ pool combine + FC ============
            for g in range(G):
                mt = wp.tile([P, T], f32, tag="mt", name="mt")
                nc.vector.tensor_tensor(
                    out=mt[:], in0=tmax[:],
                    in1=poolbsb[:, g, :], op=OP.add)
                nc.vector.tensor_reduce(out=pool_sb[:, g : g + 1], in_=mt[:],
                                        axis=mybir.AxisListType.X, op=OP.max)
            nc.sync.dma_start(poolin[:], pool_sb[:])
            tc.strict_bb_all_engine_barrier()
            nc.gpsimd.collective_compute(
                "AllReduce", OP.max, replica_groups=[core_ids],
                ins=[poolin[:]], outs=[poolout[:]])
            poolg = wp.tile([P, G], f32, tag="poolg", name="poolg")
            nc.sync.dma_start(poolg[:], poolout[:])
            fcp = psp.tile([G, 10], f32, tag="fcp", name="fcp", bufs=1)
            nc.tensor.matmul(out=fcp[:], lhsT=poolg[:], rhs=fcwsb[:],
                             start=True, stop=True)
            fco = wp.tile([G, 10], f32, tag="fco", name="fco")
            nc.vector.tensor_tensor(out=fco[:], in0=fcp[:],
                                    in1=fcbsb[:], op=OP.add)
            nc.sync.dma_start(out_t[:], fco[:])

    nc.compile()
    return nc


NEG_SLOPE_CONST = 0.2


# ----------------------------------------------------------------------------
# dispatch: cached jit(shard_map) executable with device-resident statics
# ----------------------------------------------------------------------------

class _Exec:
    """Compiled multi-core dispatcher. Static (structure-derived) inputs are
    device-resident jax arrays; per call only dynamic inputs transfer."""

    def __init__(self, nc, static_maps):
        from concourse import bass2jax as b2j
        from jax.sharding import Mesh, PartitionSpec, NamedSharding
        from jax.experimental.shard_map import shard_map

        b2j.install_neuronx_cc_hook()
        self.nc = nc
        partition_name = (nc.partition_id_tensor.name
                          if nc.partition_id_tensor else None)
        in_names, out_names, out_avals, zero_outs = [], [], [], []
        for alloc in nc.m.functions[0].allocations:
            if not isinstance(alloc, mybir.MemoryLocationSet):
                continue
            assert alloc.memorylocations
            name = alloc.memorylocations[0].name
            if alloc.kind == "ExternalInput":
                if name != partition_name:
                    in_names.append(name)
            elif alloc.kind == "ExternalOutput":
                assert alloc.tensor_shape is not None and alloc.dtype is not None
                out_names.append(name)
                shape = tuple(alloc.tensor_shape)
                dtype = mybir.dt.np(alloc.dtype)
                out_avals.append(jax.core.ShapedArray(shape, dtype))
                zero_outs.append(np.zeros((NCORES * shape[0], *shape[1:]), dtype))
        assert nc.dbg_addr is None, "debug build not supported by fast dispatch"
        n_params = len(in_names)
        self.param_names = list(in_names)
        self.out_names = list(out_names)
        self.out_avals = out_avals
        self.zero_shapes = [(z.shape, z.dtype) for z in zero_outs]
        all_names = in_names + out_names
        if partition_name is not None:
            all_names = all_names + [partition_name]

        def _body(*args):
            operands = list(args)
            if partition_name is not None:
                operands.append(b2j.partition_id_tensor())
            outs = b2j._bass_exec_p.bind(
                *operands,
                out_avals=tuple(out_avals),
                in_names=tuple(all_names),
                out_names=tuple(out_names),
                lowering_input_output_aliases=(),
                sim_require_finite=True,
                sim_require_nnan=True,
                nc=nc,
            )
            return tuple(outs)

        devices = jax.devices()[:NCORES]
        assert len(devices) == NCORES, \
            f"need {NCORES} devices, have {len(jax.devices())}"
        mesh = Mesh(np.asarray(devices), ("core",))
        self.mesh = mesh
        n_outs = len(out_names)
        donate = tuple(range(n_params, n_params + n_outs))
        self.fn = jax.jit(
            shard_map(_body, mesh=mesh,
                      in_specs=(PartitionSpec("core"),) * (n_params + n_outs),
                      out_specs=(PartitionSpec("core"),) * n_outs,
                      check_rep=False),
            donate_argnums=donate, keep_unused=True)
        sh = NamedSharding(mesh, PartitionSpec("core"))
        self.static_dev = {}
        for name in STATIC_NAMES:
            cat = np.concatenate(
                [np.asarray(static_maps[c][name]) for c in range(NCORES)], axis=0)
            self.static_dev[name] = jax.device_put(cat, sh)
        jax.block_until_ready(list(self.static_dev.values()))

    def __call__(self, dyn_maps):
        args = []
        for name in self.param_names:
            if name in self.static_dev:
                args.append(self.static_dev[name])
            else:
                args.append(np.concatenate(
                    [np.asarray(dyn_maps[c][name]) for c in range(NCORES)], axis=0))
        zeros = [np.zeros(s, d) for s, d in self.zero_shapes]
        outs = self.fn(*args, *zeros)
        # Issue the D2H request immediately: the axon client pipelines it
        # behind the execute, hiding the fetch RPC latency (~70ms) under the
        # device execution instead of paying it serially afterwards.
        if not os.environ.get("GAT_SYNCFETCH"):
            for o in outs:
                try:
                    o.copy_to_host_async()
                except Exception:
                    pass
        return {name: np.asarray(outs[i]).reshape(NCORES, *self.out_avals[i].shape)
                for i, name in enumerate(self.out_names)}


# ----------------------------------------------------------------------------
# entry point
# ----------------------------------------------------------------------------

_CACHE = {}


def _get_exec(x, edge_index, batch, G, SUPS, CH):
    import zlib
    key = (x.shape, edge_index.shape, G, CH, tuple(sorted(SUPS.items())),
           zlib.crc32(np.ascontiguousarray(edge_index).tobytes()),
           zlib.crc32(np.ascontiguousarray(batch).tobytes()))
    ent = _CACHE.get(key)
    if ent is None:
        cfg, data, aux = preprocess(np.asarray(x), np.asarray(edge_index),
                                    np.asarray(batch), G, SUPS, CH)
        nc = build(cfg)
        ex = _Exec(nc, data)
        ent = (cfg, data, aux, nc, ex)
        _CACHE.clear()
        _CACHE[key] = ent
    return ent


def run_gat(x, edge_index, batch, prm, G, SUPS=None, CH=32768):
    if SUPS is None:
        SUPS = {1: 2, 2: 2, 3: 2}
    x = np.asarray(x)
    edge_index = np.asarray(edge_index)
    batch = np.asarray(batch)
    if os.environ.get("GAT_SIM"):
        cfg, data, aux = preprocess(x, edge_index, batch, G, SUPS, CH)
        nc = build(cfg)
        dyn = host_params(x, prm, cfg, data, aux)
        from concourse.bass_interp import MultiCoreSim
        sim = MultiCoreSim(nc, num_cores=NCORES, require_finite=False,
                           require_nnan=False)
        cores = list(sim.cores.values())
        for j, cs in enumerate(cores):
            for name, val in {**data[j], **dyn[j]}.items():
                cs.tensor(name)[:] = val
        sim.simulate(check_with_hw=False)
        return np.asarray(cores[0].tensor("out"), np.float32)
    cfg, data, aux, nc, ex = _get_exec(x, edge_index, batch, G, SUPS, CH)
    dyn = host_params(x, prm, cfg, data, aux)
    import time as _t
    t0 = _t.time()
    res = ex(dyn)
    run_gat.last_spmd_wall = _t.time() - t0
    run_gat_cached.last_spmd_wall = run_gat.last_spmd_wall
    return np.asarray(res["out"][0], np.float32)


def run_gat_cached(x, edge_index, batch, prm, G, SUPS=None, CH=32768):
    return run_gat(x, edge_index, batch, prm, G, SUPS=SUPS, CH=CH)


def kernel(**inputs):
    x = np.asarray(inputs["x"], np.float32)
    edge_index = np.asarray(inputs["edge_index"], np.int64)
    batch = np.asarray(inputs["batch"], np.int64)
    prm = {k: np.asarray(v, np.float32) for k, v in inputs.items()
           if k not in ("x", "edge_index", "batch")}
    return run_gat_cached(x, edge_index, batch, prm, G=64)


# revision 14
# speedup vs baseline: 243.1749x; 1.1313x over previous
"""3-layer 2-head GAT + BatchNorm/ReLU + per-graph max-pool + FC on 8 trn2 NeuronCores.

Sharding: graph/data-parallel over dst nodes. Host relabels nodes graph-major
(degree-profile sorted within each graph, padded to graph-aligned 128-node
tiles), packs graphs onto cores, and builds per-core ELL gather schedules with
a cross-core-uniform static shape (all per-core variation is data).

Device per layer: bf16 table rows [in_features | al_src] (256B rows, one row
per node id) are gathered per edge-slot with dma_gather (int16 indices =>
32768-row windows); attention scores/softmax and the weighted feature sum run
on the vector/scalar engines along the ELL free axis; the conv matmul
(agg @ W) runs per-tile on the PE after a transpose. BN stats go through a
tiny AllReduce; next-layer features are replicated with an AllGather of bf16
shard rows; the final per-graph max pool uses an AllReduce(max).

Dispatch: a cached jit(shard_map) executable with the large call-invariant
inputs (gather schedules, masks, pool bias) resident on device; per call only
the node features (per-core [T,P,4] shard) and the small weight tensors are
uploaded. The layer-1 feature table is built on device from the x shard
(transpose+matmul for the attention logits, zero-padded bf16 rows) and
replicated with the same AllGather the later layers use.
"""
import os
import numpy as np
import ml_dtypes

import jax
import concourse.bass as bass
import concourse.bacc as bacc
import concourse.mybir as mybir
import concourse.tile as tile
from concourse._compat import cdiv, get_trn_type
from concourse.library_config import mlp

P = 128
NCORES = 8
NEG = -1.0e30
EPS = 1e-5
f32 = mybir.dt.float32
bf16 = mybir.dt.bfloat16
i16 = mybir.dt.int16
AF = mybir.ActivationFunctionType
OP = mybir.AluOpType

# (layer, Fin, C-per-head, HC=2C)
LAYERS = [(1, 3, 16, 32), (2, 32, 32, 64), (3, 64, 64, 128)]

# call-invariant inputs (functions of edge_index/batch/graph structure only):
# uploaded to device once per preprocessing cache entry.
STATIC_NAMES = ("gidx1", "gidx2", "gidx3", "nodemask", "negb", "poolbias",
                "ident")


def param_layout(G):
    """(name, shape) of every small weight tensor packed into the pblob, in
    blob order. Shared by build() and host_params()."""
    return [("w1", (3, 32)), ("w2", (32, 64)), ("w3", (64, 128)),
            ("avec1", (4, 4)), ("avec2", (32, 4)), ("avec3", (64, 4)),
            ("gam1", (1, 32)), ("bet1", (1, 32)),
            ("gam2", (1, 64)), ("bet2", (1, 64)),
            ("gam3", (1, 128)), ("bet3", (1, 128)),
            ("fcw", (128, 10)), ("fcb", (G, 10))]


def wrap_idxs(flat):
    n = len(flat)
    assert n % 16 == 0
    a = flat.astype(np.int16).reshape(n // 16, 16).T
    return np.ascontiguousarray(np.tile(a, (8, 1)))


# ----------------------------------------------------------------------------
# host preprocessing
# ----------------------------------------------------------------------------

def preprocess(x, edge_index, batch, G, SUPS, CH):
    N = x.shape[0]
    src = np.concatenate([edge_index[0], np.arange(N)]).astype(np.int64)
    dst = np.concatenate([edge_index[1], np.arange(N)]).astype(np.int64)
    batch = np.asarray(batch).astype(np.int64)
    counts = np.bincount(batch, minlength=G)
    gstart = np.concatenate([[0], np.cumsum(counts)])

    nw0 = max(cdiv(N, CH), 1)
    prof = np.zeros((N, nw0), np.int64)
    np.add.at(prof, (dst, src // CH), 1)
    order = np.empty(N, np.int64)
    for g in range(G):
        s, e = int(gstart[g]), int(gstart[g + 1])
        idx = np.arange(s, e)
        key = np.lexsort(tuple(prof[s:e, c] for c in range(nw0 - 1, -1, -1)))
        order[s:e] = idx[key]

    tiles = []
    for g in range(G):
        s, e = int(gstart[g]), int(gstart[g + 1])
        ids = order[s:e]
        nt = cdiv(max(len(ids), 1), P)
        pad = np.full(nt * P, -1, np.int64)
        pad[: len(ids)] = ids
        for t in range(nt):
            tiles.append((g, pad[t * P : (t + 1) * P]))

    by_g = {}
    for g, arr in tiles:
        by_g.setdefault(g, []).append(arr)
    core_tiles = [[] for _ in range(NCORES)]
    loads = [0] * NCORES
    for g in sorted(by_g, key=lambda g: -len(by_g[g])):
        j = int(np.argmin(loads))
        for arr in by_g[g]:
            core_tiles[j].append((g, arr))
        loads[j] += len(by_g[g])
    SUPMAX = max(SUPS.values())
    T = cdiv(max(loads), SUPMAX) * SUPMAX
    for j in range(NCORES):
        while len(core_tiles[j]) < T:
            core_tiles[j].append((-1, np.full(P, -1, np.int64)))
    NT = NCORES * T * P
    NWIN = cdiv(NT, CH)

    def assign(ct):
        o2n = np.full(N, -1, np.int64)
        n2o = np.full(NT, -1, np.int64)
        for j in range(NCORES):
            for i, (g, arr) in enumerate(ct[j]):
                base = (j * T + i) * P
                r = arr >= 0
                o2n[arr[r]] = base + np.nonzero(r)[0]
                n2o[base : base + P][r] = arr[r]
        return o2n, n2o

    def k_of(o2n):
        sn, dn = o2n[src], o2n[dst]
        cnt = np.bincount(dn * NWIN + sn // CH, minlength=NT * NWIN).reshape(NT, NWIN)
        return sn, dn, cnt.reshape(NCORES, T, P, NWIN).max(axis=2)

    o2n, n2o = assign(core_tiles)

    # Re-sort nodes within each graph by their FINAL-window in-degree profile
    # (graph->core packing moved graphs to entirely different id windows, so
    # the original-order profile sort is stale). Converges in one iteration:
    # within-graph reordering moves ids by < graph size << window size.
    for _ in range(2):
        prof2 = np.zeros((N, NWIN), np.int64)
        sn = o2n[src]
        np.add.at(prof2, (dst, sn // CH), 1)
        for j in range(NCORES):
            by_graph = {}
            for i, (g, arr) in enumerate(core_tiles[j]):
                by_graph.setdefault(g, []).append(i)
            for g, idxs_t in by_graph.items():
                if g < 0:
                    continue
                ids = np.concatenate([core_tiles[j][i][1] for i in idxs_t])
                ids = ids[ids >= 0]
                key = np.lexsort(tuple(prof2[ids, c]
                                       for c in range(NWIN - 1, -1, -1)))
                ids = ids[key]
                pad = np.full(len(idxs_t) * P, -1, np.int64)
                pad[: len(ids)] = ids
                for n_i, i in enumerate(idxs_t):
                    core_tiles[j][i] = (g, pad[n_i * P : (n_i + 1) * P])
        o2n, n2o = assign(core_tiles)

    _, _, Kc = k_of(o2n)
    for j in range(NCORES):
        key = np.argsort(-Kc[j].sum(axis=1), kind="stable")
        core_tiles[j] = [core_tiles[j][i] for i in key]
    o2n, n2o = assign(core_tiles)
    snew, dnew, Kc = k_of(o2n)
    Kuni = Kc.max(axis=0)  # [T, NWIN]

    layer_K = {}
    for L, SUP in SUPS.items():
        ns = T // SUP
        K = np.zeros((ns, NWIN), np.int64)
        for s in range(ns):
            K[s] = Kuni[s * SUP : (s + 1) * SUP].max(axis=0)
        layer_K[L] = K

    filler = np.nonzero(n2o < 0)[0]
    pad_row = {}
    for c in range(NWIN):
        f = filler[(filler // CH) == c]
        assert len(f) > 0, f"no filler row in window {c}"
        pad_row[c] = int(f[0])

    schunk = snew // CH
    eo = np.lexsort((snew, schunk, dnew))
    ds, cs, ss = dnew[eo], schunk[eo], snew[eo]
    grp = ds * NWIN + cs
    firsts = np.ones(len(grp), bool)
    firsts[1:] = grp[1:] != grp[:-1]
    g0 = np.nonzero(firsts)[0]
    rank = np.arange(len(grp)) - np.repeat(g0, np.diff(np.concatenate([g0, [len(grp)]])))
    KMAX = max(int(Kuni.max()), 1)
    slot = np.full((NT, NWIN, KMAX), -1, np.int64)
    slot[ds, cs, rank] = ss

    data = [dict() for _ in range(NCORES)]
    gcols = {}
    for L, SUP in SUPS.items():
        K = layer_K[L]
        ns = T // SUP
        for j in range(NCORES):
            segs = []
            for s in range(ns):
                for c in range(NWIN):
                    k = int(K[s, c])
                    if k == 0:
                        continue
                    rows = slot[(j * T + s * SUP) * P : (j * T + (s + 1) * SUP) * P, c, :k]
                    sub = rows.reshape(SUP, P, k)
                    loc = np.where(sub < 0, pad_row[c], sub) - c * CH
                    assert loc.min() >= 0 and loc.max() < CH
                    segs.append(wrap_idxs(loc.transpose(0, 2, 1).reshape(-1)))
            arr = (np.concatenate(segs, axis=1) if segs
                   else np.zeros((P, 8), np.int16))
            data[j][f"gidx{L}"] = np.ascontiguousarray(arr)
        gcols[L] = data[0][f"gidx{L}"].shape[1]
        for j in range(NCORES):
            assert data[j][f"gidx{L}"].shape[1] == gcols[L]

    for j in range(NCORES):
        nm = np.zeros((T, P), np.float32)
        pb = np.full((G, T), NEG, np.float32)
        for i, (g, arr) in enumerate(core_tiles[j]):
            nm[i] = (arr >= 0).astype(np.float32)
            if g >= 0:
                pb[g, i] = 0.0
        data[j]["nodemask"] = nm
        data[j]["negb"] = ((nm - 1.0) * 1e30).astype(np.float32)
        data[j]["poolbias"] = np.ascontiguousarray(
            np.repeat(pb[:, None, :], P, axis=1).reshape(G * P, T))
        data[j]["ident"] = np.eye(128, dtype=np.float32)

    cfg = dict(N=int(N), G=int(G), T=int(T), NT=int(NT), NWIN=int(NWIN), CH=int(CH),
               SUPS=dict(SUPS), gcols=gcols,
               layer_K={L: K.astype(int) for L, K in layer_K.items()})
    aux = dict(o2n=o2n, n2o=n2o)
    return cfg, data, aux


def host_params(x, prm, cfg, data, aux):
    """Per-call (x/param-dependent) inputs: per-core x shard + small weights."""
    n2o = aux["n2o"]
    T, NT = cfg["T"], cfg["NT"]

    def avec(W, a_s, a_d, F, Hc):
        Wr = np.asarray(W).reshape(F, 2, Hc)
        vs = np.stack([Wr[:, h, :] @ np.asarray(a_s)[h] for h in range(2)], axis=1)
        vd = np.stack([Wr[:, h, :] @ np.asarray(a_d)[h] for h in range(2)], axis=1)
        return np.concatenate([vs, vd], axis=1).astype(np.float32)

    av1 = avec(prm["W1"], prm["as1"], prm["ad1"], 3, 16)
    av1p = np.zeros((4, 4), np.float32)
    av1p[0:3] = av1
    av2 = avec(prm["W2"], prm["as2"], prm["ad2"], 32, 32)
    av3 = avec(prm["W3"], prm["as3"], prm["ad3"], 64, 64)

    vals = {
        "w1": np.asarray(prm["W1"], np.float32),
        "w2": np.asarray(prm["W2"], np.float32),
        "w3": np.asarray(prm["W3"], np.float32),
        "avec1": av1p, "avec2": av2, "avec3": av3,
        "fcw": np.asarray(prm["fcw"], np.float32),
        "fcb": np.tile(np.asarray(prm["fcb"], np.float32).reshape(1, -1), (cfg["G"], 1)),
    }
    for L, F, Hc, HC in LAYERS:
        vals[f"gam{L}"] = np.asarray(prm[f"g{L}"], np.float32).reshape(1, -1)
        vals[f"bet{L}"] = np.asarray(prm[f"be{L}"], np.float32).reshape(1, -1)
    parts = []
    for name, shape in param_layout(cfg["G"]):
        v = vals[name]
        assert v.shape == shape, (name, v.shape, shape)
        parts.append(v.ravel())
    pblob = np.ascontiguousarray(np.concatenate(parts).astype(np.float32))
    xr = np.asarray(x, np.float32)
    dyn = [{"pblob": pblob} for _ in range(NCORES)]
    for j in range(NCORES):
        ids = n2o[j * T * P : (j + 1) * T * P]
        xs = np.zeros((T * P, 4), np.float32)
        r = ids >= 0
        xs[r, 0:3] = xr[ids[r]]
        dyn[j]["xs"] = xs.reshape(T, P, 4).astype(ml_dtypes.bfloat16)
    return dyn


# ----------------------------------------------------------------------------
# device program
# ----------------------------------------------------------------------------

def build(cfg):
    T, NT, NWIN, G, N, CH = (cfg["T"], cfg["NT"], cfg["NWIN"], cfg["G"],
                             cfg["N"], cfg["CH"])
    SUPS, layer_K, gcols = cfg["SUPS"], cfg["layer_K"], cfg["gcols"]
    core_ids = list(range(NCORES))

    nc = bacc.Bacc(get_trn_type() or "TRN2", target_bir_lowering=False,
                   dynamic_dma_scratch_size=int(os.environ.get("GAT_SCRATCH",
                                                               "32768")),
                   num_swdge_queues=int(os.environ.get("GAT_NQ", "2")))

    xs_t = nc.dram_tensor("xs", [T, P, 4], bf16, kind="ExternalInput")
    gidx_t = {L: nc.dram_tensor(f"gidx{L}", [P, gcols[L]], i16, kind="ExternalInput")
              for L, _, _, _ in LAYERS}
    nmask_t = nc.dram_tensor("nodemask", [T, P], f32, kind="ExternalInput")
    negb_t = nc.dram_tensor("negb", [T, P], f32, kind="ExternalInput")
    poolb_t = nc.dram_tensor("poolbias", [G * P, T], f32, kind="ExternalInput")
    playout = param_layout(G)
    NPRM = sum(a * b for _, (a, b) in playout)
    pblob_t = nc.dram_tensor("pblob", [1, NPRM], f32, kind="ExternalInput")
    pview = {}
    off = 0
    for name, (a, b) in playout:
        pview[name] = pblob_t[:, off : off + a * b].rearrange(
            "o (a b) -> (o a) b", a=a)
        off += a * b
    ident_t = nc.dram_tensor("ident", [P, P], f32, kind="ExternalInput")
    out_t = nc.dram_tensor("out", [G, 10], f32, kind="ExternalOutput")

    # internal DRAM
    tabn = {1: nc.dram_tensor("tabA", [NT, 128], bf16, addr_space="Shared"),
            2: nc.dram_tensor("tabB", [NT, 128], bf16, addr_space="Shared"),
            3: nc.dram_tensor("tabC", [NT, 128], bf16, addr_space="Shared")}
    shard = {1: nc.dram_tensor("shardA", [T * P, 128], bf16),
             2: nc.dram_tensor("shardB", [T * P, 128], bf16),
             3: nc.dram_tensor("shardC", [T * P, 128], bf16)}
    conv_t = {L: nc.dram_tensor(f"conv{L}", [T, P, HC], f32)
              for L, F, Hc, HC in LAYERS}
    stin = {L: nc.dram_tensor(f"stin{L}", [1, 2 * HC], f32)
            for L, F, Hc, HC in LAYERS}
    stout = {L: nc.dram_tensor(f"stout{L}", [1, 2 * HC], f32, addr_space="Shared")
             for L, F, Hc, HC in LAYERS}
    poolin = nc.dram_tensor("poolin", [P, G], f32)
    poolout = nc.dram_tensor("poolout", [P, G], f32, addr_space="Shared")

    with tile.TileContext(nc) as tc:
        with (
            tc.tile_pool(name="persist", bufs=1) as pp,
            tc.tile_pool(name="io", bufs=int(os.environ.get("GAT_IOBUFS", "3"))) as iop,
            tc.tile_pool(name="gath", bufs=2) as gpool,
            tc.tile_pool(name="work", bufs=int(os.environ.get("GAT_WPBUFS", "2"))) as wp,
            tc.tile_pool(name="psum", bufs=2, space="PSUM") as psp,
        ):
            nc.gpsimd.load_library(mlp)
            tc.strict_bb_all_engine_barrier()
            ident = pp.tile([P, P], f32, tag="ident", name="ident")
            nc.sync.dma_start(ident[:], ident_t[:])
            ones = pp.tile([P, 1], f32, tag="ones", name="ones")
            nc.vector.memset(ones[:], 1.0)
            onesr = pp.tile([1, P], f32, tag="onesr", name="onesr")
            nc.vector.memset(onesr[:], 1.0)

            wsb = {}
            for L, F, Hc, HC in LAYERS:
                wsb[L] = pp.tile([F, HC], f32, tag=f"w{L}", name=f"w{L}")
                nc.sync.dma_start(wsb[L][:], pview[f"w{L}"])
            avsb = {}
            for L in (1, 2, 3):
                Fin = 4 if L == 1 else LAYERS[L - 1][1]
                avsb[L] = pp.tile([Fin, 4], f32, tag=f"av{L}", name=f"av{L}")
                nc.sync.dma_start(avsb[L][:], pview[f"avec{L}"])
            gamsb, betsb = {}, {}
            for L, F, Hc, HC in LAYERS:
                gamsb[L] = pp.tile([1, HC], f32, tag=f"gam{L}", name=f"gam{L}")
                betsb[L] = pp.tile([1, HC], f32, tag=f"bet{L}", name=f"bet{L}")
                nc.sync.dma_start(gamsb[L][:], pview[f"gam{L}"])
                nc.sync.dma_start(betsb[L][:], pview[f"bet{L}"])
            fcwsb = pp.tile([128, 10], f32, tag="fcw", name="fcw")
            nc.sync.dma_start(fcwsb[:], pview["fcw"])
            fcbsb = pp.tile([G, 10], f32, tag="fcb", name="fcb")
            nc.sync.dma_start(fcbsb[:], pview["fcb"])
            poolbsb = pp.tile([P, G, T], f32, tag="poolb", name="poolb")
            nc.sync.dma_start(poolbsb[:], poolb_t[:].rearrange("(g p) t -> p g t", p=P))
            aldbuf = {L: pp.tile([P, T, 2], f32, tag=f"ald{L}", name=f"ald{L}")
                      for L in (1, 2, 3)}
            tmax = pp.tile([P, T], f32, tag="tmax", name="tmax")
            pool_sb = pp.tile([P, G], f32, tag="pool", name="pool")

            # ============ layer-1 table build (shard rows from x) ============
            SUP1 = SUPS[1]
            for s in range(T // SUP1):
                xst = iop.tile([P, SUP1, 4], bf16, tag="xst", name="xst")
                nc.sync.dma_start(
                    xst[:], xs_t[s * SUP1 : (s + 1) * SUP1].rearrange("t p f -> p t f"))
                mkb = iop.tile([P, SUP1], f32, tag="mkb0", name="mkb0")
                nc.sync.dma_start(
                    mkb[:], nmask_t[s * SUP1 : (s + 1) * SUP1].rearrange("t p -> p t"))
                ngb = iop.tile([P, SUP1], f32, tag="ngb0", name="ngb0")
                nc.sync.dma_start(
                    ngb[:], negb_t[s * SUP1 : (s + 1) * SUP1].rearrange("t p -> p t"))
                rbc = wp.tile([P, SUP1, 128], bf16, tag="rbc", name="rbc")
                nc.vector.memset(rbc[:], 0.0)
                nc.vector.tensor_copy(out=rbc[:, :, 0:3], in_=xst[:, :, 0:3])
                xstf = wp.tile([P, SUP1, 4], f32, tag="xstf", name="xstf")
                nc.vector.tensor_copy(out=xstf[:], in_=xst[:])
                for t in range(SUP1):
                    ti = s * SUP1 + t
                    tp0 = psp.tile([4, P], f32, tag="tp", name="tp0")
                    nc.tensor.transpose(out=tp0[:], in_=xstf[:, t, :],
                                        identity=ident[:])
                    xT = wp.tile([4, P], f32, tag="xT", name="xT")
                    nc.vector.tensor_copy(out=xT[:], in_=tp0[:])
                    ps4 = psp.tile([P, 4], f32, tag="ps4", name="ps40", bufs=1)
                    nc.tensor.matmul(out=ps4[:], lhsT=xT[:], rhs=avsb[1][:],
                                     start=True, stop=True)
                    alsb = wp.tile([P, 2], f32, tag="alsb0", name="alsb0")
                    nc.vector.scalar_tensor_tensor(
                        out=alsb[:], in0=ps4[:, 0:2],
                        scalar=mkb[:, t : t + 1],
                        in1=ngb[:, t : t + 1].to_broadcast([P, 2]),
                        op0=OP.mult, op1=OP.add)
                    nc.vector.tensor_copy(out=aldbuf[1][:, ti], in_=ps4[:, 2:4])
                    nc.vector.tensor_copy(out=rbc[:, t, 3:5], in_=alsb[:])
                nc.sync.dma_start(
                    shard[1][s * SUP1 * P : (s + 1) * SUP1 * P, :]
                    .rearrange("(t p) f -> p t f", p=P), rbc[:])
            tc.strict_bb_all_engine_barrier()
            nc.gpsimd.collective_compute(
                "AllGather", OP.bypass, replica_groups=[core_ids],
                ins=[shard[1][:]], outs=[tabn[1][:]])
            tc.strict_bb_all_engine_barrier()

            for L, F, Hc, HC in LAYERS:
                SUP = SUPS[L]
                K = layer_K[L]
                ns = T // SUP
                tab_ap = tabn[L]
                conv = conv_t[L]

                # ============ edge phase ============
                gofs = 0
                for s in range(ns):
                    Ks = [int(K[s, c]) for c in range(NWIN)]
                    S = sum(Ks)
                    if S == 0:
                        cvz = wp.tile([P, SUP, HC], f32, tag="cvz", name="cvz")
                        nc.vector.memset(cvz[:], 0.0)
                        nc.sync.dma_start(
                            conv[s * SUP : (s + 1) * SUP].rearrange("t p f -> p t f"),
                            cvz[:])
                        continue
                    gsb = iop.tile([P, 8 * SUP * S], i16, tag="gsb", name="gsb")
                    nc.sync.dma_start(gsb[:], gidx_t[L][:, gofs : gofs + 8 * SUP * S])
                    gofs += 8 * SUP * S
                    gt = gpool.tile([P, SUP * S, 128], bf16, tag="gt", name="gt",
                                    bufs=int(os.environ.get("GAT_GTBUFS", "2")))
                    so = 0
                    CAPC = int(os.environ.get('GAT_CAPC', '8'))  # 1024-idx HW limit
                    NQ = int(os.environ.get("GAT_NQ", "2"))
                    qn = 0
                    for c in range(NWIN):
                        k = Ks[c]
                        if k == 0:
                            continue
                        win = tab_ap[c * CH : min(c * CH + CH, NT), :]
                        base = SUP * so
                        tot = SUP * k
                        for ofs in range(0, tot, CAPC):
                            w = min(CAPC, tot - ofs)
                            nidx = w * P
                            nc.gpsimd.dma_gather(
                                gt[:, base + ofs : base + ofs + w, :], win,
                                gsb[:, 8 * (base + ofs) : 8 * (base + ofs + w)],
                                nidx, nidx, 128, queue_num=qn % NQ)
                            qn += 1
                        so += k

                    ald_ap = aldbuf[L][:, s * SUP : (s + 1) * SUP, :]

                    scr = wp.tile([P, SUP, S, 2], f32, tag="scr", name="scr")
                    so = 0
                    for c in range(NWIN):
                        k = Ks[c]
                        if k == 0:
                            continue
                        in0 = gt[:, SUP * so : SUP * (so + k), F : F + 2]
                        in0 = in0.rearrange("p (t k) h -> p t k h", k=k)
                        in1 = ald_ap.unsqueeze(2).to_broadcast([P, SUP, k, 2])
                        nc.vector.tensor_tensor(
                            out=scr[:, :, so : so + k, :], in0=in0, in1=in1,
                            op=OP.add)
                        so += k
                    ex = wp.tile([P, SUP, S, 2], f32, tag="ex", name="ex")
                    nc.vector.tensor_scalar_mul(out=ex[:], in0=scr[:],
                                                scalar1=NEG_SLOPE_CONST)
                    nc.vector.tensor_tensor(out=ex[:], in0=ex[:], in1=scr[:],
                                            op=OP.max)
                    nc.scalar.activation(out=ex[:], in_=ex[:], func=AF.Exp)
                    den = wp.tile([P, SUP, 2], f32, tag="den", name="den")
                    nc.vector.tensor_reduce(
                        out=den[:], in_=ex[:].rearrange("p t s h -> p t h s"),
                        axis=mybir.AxisListType.X, op=OP.add)
                    nc.vector.tensor_scalar_max(out=den[:], in0=den[:], scalar1=1e-30)
                    rden = wp.tile([P, SUP, 2], f32, tag="rden", name="rden")
                    nc.vector.reciprocal(rden[:], den[:])
                    alph = wp.tile([P, SUP, S, 2], bf16, tag="alph", name="alph")
                    nc.vector.tensor_tensor(
                        out=alph[:], in0=ex[:],
                        in1=rden[:].unsqueeze(2).to_broadcast([P, SUP, S, 2]),
                        op=OP.mult)
                    # tmp layout [P, t, h, s, F]: multiply fully contiguous
                    # (inner f stride 1 on all streams); the single-stream
                    # reduce pays the stride instead.
                    tmp = wp.tile([P, SUP, 2, S, F], bf16, tag="tmp", name="tmp",
                                  bufs=int(os.environ.get("GAT_TMPBUFS", "1")))
                    so = 0
                    for c in range(NWIN):
                        k = Ks[c]
                        if k == 0:
                            continue
                        in0 = gt[:, SUP * so : SUP * (so + k), 0:F]
                        in0 = in0.rearrange("p (t k) f -> p t k f", k=k)
                        for h in range(2):
                            in1 = alph[:, :, so : so + k, h : h + 1]
                            in1 = in1.to_broadcast([P, SUP, k, F])
                            nc.vector.tensor_tensor(
                                out=tmp[:, :, h, so : so + k, :], in0=in0,
                                in1=in1, op=OP.mult)
                        so += k
                    agg = wp.tile([P, SUP, 2, F], f32, tag="agg", name="agg")
                    nc.vector.tensor_reduce(
                        out=agg[:].rearrange("p t h f -> p (t h) f"),
                        in_=tmp[:].rearrange("p t h s f -> p (t h) f s"),
                        axis=mybir.AxisListType.X, op=OP.add)
                    for t in range(SUP):
                        ti = s * SUP + t
                        cvp = psp.tile([P, HC], f32, tag="cvp", name="cvp")
                        for h in range(2):
                            tp = psp.tile([F, P], f32, tag="tp", name="tp")
                            nc.tensor.transpose(
                                out=tp[:], in_=agg[:, t, h, :],
                                identity=ident[:])
                            aggT = wp.tile([F, P], f32, tag="aggT", name="aggT")
                            nc.vector.tensor_copy(out=aggT[:], in_=tp[:])
                            nc.tensor.matmul(
                                out=cvp[:, h * Hc : (h + 1) * Hc],
                                lhsT=aggT[:],
                                rhs=wsb[L][:, h * Hc : (h + 1) * Hc],
                                start=True, stop=True)
                        cvs = wp.tile([P, HC], f32, tag="cvs", name="cvs")
                        nc.vector.tensor_copy(out=cvs[:], in_=cvp[:])
                        nc.sync.dma_start(conv[ti], cvs[:])

                tc.strict_bb_all_engine_barrier()

                # ============ stats ============
                acc = pp.tile([P, 2 * HC], f32, tag=f"acc{L}", name=f"acc{L}")
                nc.vector.memset(acc[:], 0.0)
                TB = SUP
                for b in range(T // TB):
                    cvb = iop.tile([P, TB, HC], f32, tag="cvb", name="cvb")
                    nc.sync.dma_start(
                        cvb[:], conv[b * TB : (b + 1) * TB].rearrange("t p f -> p t f"))
                    mkb = iop.tile([P, TB], f32, tag="mkb", name="mkb")
                    nc.sync.dma_start(
                        mkb[:], nmask_t[b * TB : (b + 1) * TB].rearrange("t p -> p t"))
                    cvm = wp.tile([P, TB, HC], f32, tag="cvm", name="cvm")
                    nc.vector.tensor_tensor(
                        out=cvm[:], in0=cvb[:],
                        in1=mkb[:].unsqueeze(2).to_broadcast([P, TB, HC]), op=OP.mult)
                    sq = wp.tile([P, TB, HC], f32, tag="sqt", name="sqt")
                    nc.vector.tensor_tensor(out=sq[:], in0=cvm[:], in1=cvb[:],
                                            op=OP.mult)
                    r1 = wp.tile([P, HC], f32, tag="r1", name="r1")
                    nc.vector.tensor_reduce(
                        out=r1[:], in_=cvm[:].rearrange("p t f -> p f t"),
                        axis=mybir.AxisListType.X, op=OP.add)
                    nc.vector.tensor_tensor(out=acc[:, 0:HC], in0=acc[:, 0:HC],
                                            in1=r1[:], op=OP.add)
                    r2 = wp.tile([P, HC], f32, tag="r2", name="r2")
                    nc.vector.tensor_reduce(
                        out=r2[:], in_=sq[:].rearrange("p t f -> p f t"),
                        axis=mybir.AxisListType.X, op=OP.add)
                    nc.vector.tensor_tensor(out=acc[:, HC:], in0=acc[:, HC:],
                                            in1=r2[:], op=OP.add)
                stp = psp.tile([1, 2 * HC], f32, tag="stp", name="stp", bufs=1)
                nc.tensor.matmul(out=stp[:], lhsT=ones[:], rhs=acc[:],
                                 start=True, stop=True)
                sts = wp.tile([1, 2 * HC], f32, tag="sts", name="sts")
                nc.vector.tensor_copy(out=sts[:], in_=stp[:])
                nc.sync.dma_start(stin[L][:], sts[:])
                tc.strict_bb_all_engine_barrier()
                nc.gpsimd.collective_compute(
                    "AllReduce", OP.add, replica_groups=[core_ids],
                    ins=[stin[L][:]], outs=[stout[L][:]])
                stg = wp.tile([1, 2 * HC], f32, tag="stg", name="stg")
                nc.sync.dma_start(stg[:], stout[L][:])
                mu = wp.tile([1, HC], f32, tag="mu", name="mu")
                nc.vector.tensor_scalar_mul(out=mu[:], in0=stg[:, 0:HC],
                                            scalar1=1.0 / N)
                var = wp.tile([1, HC], f32, tag="var", name="var")
                nc.vector.tensor_scalar_mul(out=var[:], in0=stg[:, HC:],
                                            scalar1=1.0 / N)
                mu2 = wp.tile([1, HC], f32, tag="mu2", name="mu2")
                nc.vector.tensor_tensor(out=mu2[:], in0=mu[:], in1=mu[:], op=OP.mult)
                nc.vector.tensor_tensor(out=var[:], in0=var[:], in1=mu2[:],
                                        op=OP.subtract)
                nc.vector.tensor_scalar_add(out=var[:], in0=var[:], scalar1=EPS)
                sd = wp.tile([1, HC], f32, tag="sd", name="sd")
                nc.scalar.activation(out=sd[:], in_=var[:], func=AF.Sqrt)
                rsd = wp.tile([1, HC], f32, tag="rsd", name="rsd")
                nc.vector.reciprocal(rsd[:], sd[:])
                acst = wp.tile([1, 2 * HC], f32, tag="acst", name="acst")
                asc = acst[:, 0:HC]
                csc = acst[:, HC:]
                nc.vector.tensor_tensor(out=asc, in0=gamsb[L][:], in1=rsd[:],
                                        op=OP.mult)
                nc.vector.tensor_tensor(out=csc, in0=mu[:], in1=asc, op=OP.mult)
                nc.vector.tensor_tensor(out=csc, in0=betsb[L][:], in1=csc,
                                        op=OP.subtract)
                bcp = psp.tile([P, 2 * HC], f32, tag="bcp", name="bcp", bufs=1)
                nc.tensor.matmul(out=bcp[:], lhsT=onesr[:], rhs=acst[:],
                                 start=True, stop=True)
                bcs = pp.tile([P, 2 * HC], f32, tag=f"bcs{L}", name=f"bcs{L}")
                nc.vector.tensor_copy(out=bcs[:], in_=bcp[:])

                # ============ BN + next-layer table / pooling ============
                SUP2 = SUP
                for s2 in range(T // SUP2):
                    cvb = iop.tile([P, SUP2, HC], f32, tag="cvb2", name="cvb2")
                    nc.sync.dma_start(
                        cvb[:],
                        conv[s2 * SUP2 : (s2 + 1) * SUP2].rearrange("t p f -> p t f"))
                    mkb = iop.tile([P, SUP2], f32, tag="mkb2", name="mkb2")
                    nc.sync.dma_start(
                        mkb[:],
                        nmask_t[s2 * SUP2 : (s2 + 1) * SUP2].rearrange("t p -> p t"))
                    ngb = iop.tile([P, SUP2], f32, tag="ngb2", name="ngb2")
                    nc.sync.dma_start(
                        ngb[:],
                        negb_t[s2 * SUP2 : (s2 + 1) * SUP2].rearrange("t p -> p t"))
                    inp = wp.tile([P, SUP2, HC], f32, tag="inp", name="inp")
                    a_b = bcs[:, 0:HC].unsqueeze(1).to_broadcast([P, SUP2, HC])
                    nc.vector.tensor_tensor(out=inp[:], in0=cvb[:], in1=a_b,
                                            op=OP.mult)
                    c_b = bcs[:, HC:].unsqueeze(1).to_broadcast([P, SUP2, HC])
                    nc.vector.tensor_tensor(out=inp[:], in0=inp[:], in1=c_b,
                                            op=OP.add)
                    nc.vector.tensor_scalar_max(out=inp[:], in0=inp[:], scalar1=0.0)
                    if L < 3:
                        nc.vector.tensor_tensor(
                            out=inp[:], in0=inp[:],
                            in1=mkb[:].unsqueeze(2).to_broadcast([P, SUP2, HC]),
                            op=OP.mult)
                        for t2 in range(SUP2):
                            ti = s2 * SUP2 + t2
                            tp2 = psp.tile([HC, P], f32, tag="tp", name="tp2")
                            nc.tensor.transpose(out=tp2[:], in_=inp[:, t2],
                                                identity=ident[:])
                            inT = wp.tile([HC, P], f32, tag="inT", name="inT")
                            nc.vector.tensor_copy(out=inT[:], in_=tp2[:])
                            ps4 = psp.tile([P, 4], f32, tag="ps4", name="ps4", bufs=1)
                            nc.tensor.matmul(out=ps4[:], lhsT=inT[:],
                                             rhs=avsb[L + 1][:], start=True, stop=True)
                            alsb = wp.tile([P, 2], f32, tag="alsb", name="alsb")
                            nc.vector.scalar_tensor_tensor(
                                out=alsb[:], in0=ps4[:, 0:2],
                                scalar=mkb[:, t2 : t2 + 1],
                                in1=ngb[:, t2 : t2 + 1].to_broadcast([P, 2]),
                                op0=OP.mult, op1=OP.add)
                            nc.vector.tensor_copy(out=aldbuf[L + 1][:, ti],
                                                  in_=ps4[:, 2:4])
                            rb = wp.tile([P, 128], bf16, tag="rb", name="rb")
                            nc.vector.memset(rb[:], 0.0)
                            nc.vector.tensor_copy(out=rb[:, 0:HC], in_=inp[:, t2])
                            nc.vector.tensor_copy(out=rb[:, HC : HC + 2], in_=alsb[:])
                            nc.sync.dma_start(
                                shard[L + 1][ti * P : (ti + 1) * P, :], rb[:])
                    else:
                        for t2 in range(SUP2):
                            ti = s2 * SUP2 + t2
                            h3g = wp.tile([P, HC], f32, tag="h3g", name="h3g")
                            nc.vector.scalar_tensor_tensor(
                                out=h3g[:], in0=inp[:, t2],
                                scalar=mkb[:, t2 : t2 + 1],
                                in1=ngb[:, t2 : t2 + 1].to_broadcast([P, HC]),
                                op0=OP.mult, op1=OP.add)
                            tp3 = psp.tile([HC, P], f32, tag="tp", name="tp3")
                            nc.tensor.transpose(out=tp3[:], in_=h3g[:],
                                                identity=ident[:])
                            nc.vector.tensor_reduce(
                                out=tmax[:, ti : ti + 1], in_=tp3[:],
                                axis=mybir.AxisListType.X, op=OP.max)
                if L < 3:
                    tc.strict_bb_all_engine_barrier()
                    nc.gpsimd.collective_compute(
                        "AllGather", OP.bypass, replica_groups=[core_ids],
                        ins=[shard[L + 1][:]], outs=[tabn[L + 1][:]])
                    tc.strict_bb_all_engine_barrier()

            # ============ pool combine + FC ============
            for g in range(G):
                mt = wp.tile([P, T], f32, tag="mt", name="mt")
                nc.vector.tensor_tensor(
                    out=mt[:], in0=tmax[:],
                    in1=poolbsb[:, g, :], op=OP.add)
                nc.vector.tensor_reduce(out=pool_sb[:, g : g + 1], in_=mt[:],
                                        axis=mybir.AxisListType.X, op=OP.max)
            nc.sync.dma_start(poolin[:], pool_sb[:])
            tc.strict_bb_all_engine_barrier()
            nc.gpsimd.collective_compute(
                "AllReduce", OP.max, replica_groups=[core_ids],
                ins=[poolin[:]], outs=[poolout[:]])
            poolg = wp.tile([P, G], f32, tag="poolg", name="poolg")
            nc.sync.dma_start(poolg[:], poolout[:])
            fcp = psp.tile([G, 10], f32, tag="fcp", name="fcp", bufs=1)
            nc.tensor.matmul(out=fcp[:], lhsT=poolg[:], rhs=fcwsb[:],
                             start=True, stop=True)
            fco = wp.tile([G, 10], f32, tag="fco", name="fco")
            nc.vector.tensor_tensor(out=fco[:], in0=fcp[:],
                                    in1=fcbsb[:], op=OP.add)
            nc.sync.dma_start(out_t[:], fco[:])

    nc.compile()
    return nc


NEG_SLOPE_CONST = 0.2


# ----------------------------------------------------------------------------
# dispatch: cached jit(shard_map) executable with device-resident statics
# ----------------------------------------------------------------------------

class _Exec:
    """Compiled multi-core dispatcher. Static (structure-derived) inputs are
    device-resident jax arrays; per call only dynamic inputs transfer."""

    def __init__(self, nc, static_maps):
        from concourse import bass2jax as b2j
        from jax.sharding import Mesh, PartitionSpec, NamedSharding
        from jax.experimental.shard_map import shard_map

        b2j.install_neuronx_cc_hook()
        self.nc = nc
        partition_name = (nc.partition_id_tensor.name
                          if nc.partition_id_tensor else None)
        in_names, out_names, out_avals, zero_outs = [], [], [], []
        for alloc in nc.m.functions[0].allocations:
            if not isinstance(alloc, mybir.MemoryLocationSet):
                continue
            assert alloc.memorylocations
            name = alloc.memorylocations[0].name
            if alloc.kind == "ExternalInput":
                if name != partition_name:
                    in_names.append(name)
            elif alloc.kind == "ExternalOutput":
                assert alloc.tensor_shape is not None and alloc.dtype is not None
                out_names.append(name)
                shape = tuple(alloc.tensor_shape)
                dtype = mybir.dt.np(alloc.dtype)
                out_avals.append(jax.core.ShapedArray(shape, dtype))
                zero_outs.append(np.zeros((NCORES * shape[0], *shape[1:]), dtype))
        assert nc.dbg_addr is None, "debug build not supported by fast dispatch"
        n_params = len(in_names)
        self.param_names = list(in_names)
        self.out_names = list(out_names)
        self.out_avals = out_avals
        self.zero_shapes = [(z.shape, z.dtype) for z in zero_outs]
        all_names = in_names + out_names
        if partition_name is not None:
            all_names = all_names + [partition_name]

        def _body(*args):
            operands = list(args)
            if partition_name is not None:
                operands.append(b2j.partition_id_tensor())
            outs = b2j._bass_exec_p.bind(
                *operands,
                out_avals=tuple(out_avals),
                in_names=tuple(all_names),
                out_names=tuple(out_names),
                lowering_input_output_aliases=(),
                sim_require_finite=True,
                sim_require_nnan=True,
                nc=nc,
            )
            return tuple(outs)

        devices = jax.devices()[:NCORES]
        assert len(devices) == NCORES, \
            f"need {NCORES} devices, have {len(jax.devices())}"
        mesh = Mesh(np.asarray(devices), ("core",))
        self.mesh = mesh
        n_outs = len(out_names)
        donate = tuple(range(n_params, n_params + n_outs))
        self.fn = jax.jit(
            shard_map(_body, mesh=mesh,
                      in_specs=(PartitionSpec("core"),) * (n_params + n_outs),
                      out_specs=(PartitionSpec("core"),) * n_outs,
                      check_rep=False),
            donate_argnums=donate, keep_unused=True)
        sh = NamedSharding(mesh, PartitionSpec("core"))
        self.static_dev = {}
        for name in STATIC_NAMES:
            cat = np.concatenate(
                [np.asarray(static_maps[c][name]) for c in range(NCORES)], axis=0)
            self.static_dev[name] = jax.device_put(cat, sh)
        jax.block_until_ready(list(self.static_dev.values()))

    def __call__(self, dyn_maps):
        args = []
        for name in self.param_names:
            if name in self.static_dev:
                args.append(self.static_dev[name])
            else:
                args.append(np.concatenate(
                    [np.asarray(dyn_maps[c][name]) for c in range(NCORES)], axis=0))
        zeros = [np.zeros(s, d) for s, d in self.zero_shapes]
        outs = self.fn(*args, *zeros)
        # GAT_ASYNCFETCH pipelines the D2H request behind the execute. It
        # measured slightly faster when uploads were the bottleneck, but was
        # implicated in an NRT_EXEC_UNIT_UNRECOVERABLE wedge at full size, so
        # the safe synchronous fetch is the default.
        if os.environ.get("GAT_ASYNCFETCH"):
            for o in outs:
                try:
                    o.copy_to_host_async()
                except Exception:
                    pass
        return {name: np.asarray(outs[i]).reshape(NCORES, *self.out_avals[i].shape)
                for i, name in enumerate(self.out_names)}


# ----------------------------------------------------------------------------
# entry point
# ----------------------------------------------------------------------------

_CACHE = {}


def _get_exec(x, edge_index, batch, G, SUPS, CH):
    import zlib
    key = (x.shape, edge_index.shape, G, CH, tuple(sorted(SUPS.items())),
           zlib.crc32(np.ascontiguousarray(edge_index).tobytes()),
           zlib.crc32(np.ascontiguousarray(batch).tobytes()))
    ent = _CACHE.get(key)
    if ent is None:
        cfg, data, aux = preprocess(np.asarray(x), np.asarray(edge_index),
                                    np.asarray(batch), G, SUPS, CH)
        nc = build(cfg)
        ex = _Exec(nc, data)
        ent = (cfg, data, aux, nc, ex)
        _CACHE.clear()
        _CACHE[key] = ent
    return ent


def run_gat(x, edge_index, batch, prm, G, SUPS=None, CH=32768):
    if SUPS is None:
        SUPS = {1: 2, 2: 2, 3: 2}
    x = np.asarray(x)
    edge_index = np.asarray(edge_index)
    batch = np.asarray(batch)
    if os.environ.get("GAT_SIM"):
        cfg, data, aux = preprocess(x, edge_index, batch, G, SUPS, CH)
        nc = build(cfg)
        dyn = host_params(x, prm, cfg, data, aux)
        from concourse.bass_interp import MultiCoreSim
        sim = MultiCoreSim(nc, num_cores=NCORES, require_finite=False,
                           require_nnan=False)
        cores = list(sim.cores.values())
        for j, cs in enumerate(cores):
            for name, val in {**data[j], **dyn[j]}.items():
                cs.tensor(name)[:] = val
        sim.simulate(check_with_hw=False)
        return np.asarray(cores[0].tensor("out"), np.float32)
    cfg, data, aux, nc, ex = _get_exec(x, edge_index, batch, G, SUPS, CH)
    dyn = host_params(x, prm, cfg, data, aux)
    import time as _t
    t0 = _t.time()
    res = ex(dyn)
    run_gat.last_spmd_wall = _t.time() - t0
    run_gat_cached.last_spmd_wall = run_gat.last_spmd_wall
    return np.asarray(res["out"][0], np.float32)


def run_gat_cached(x, edge_index, batch, prm, G, SUPS=None, CH=32768):
    return run_gat(x, edge_index, batch, prm, G, SUPS=SUPS, CH=CH)


def kernel(**inputs):
    x = np.asarray(inputs["x"], np.float32)
    edge_index = np.asarray(inputs["edge_index"], np.int64)
    batch = np.asarray(inputs["batch"], np.int64)
    prm = {k: np.asarray(v, np.float32) for k, v in inputs.items()
           if k not in ("x", "edge_index", "batch")}
    return run_gat_cached(x, edge_index, batch, prm, G=64)
